# revision 1
# baseline (speedup 1.0000x reference)
"""Accurate SDF (garment-to-body signed distance) on 8 Trainium2 cores.

Strategy: the O(G*F) closest-point-on-triangle + argmin (the retrieval/KNN
core) runs on device; faces are sharded 8 ways (1722 faces/core, padded to
1792 = 14*128), every core sees all 2*1024 garment points. Host does the
O(F) index gathers / per-face constants up front and the O(G) winner
post-processing (region code, normals, sign) at the end, then merges the
8 per-core argmin candidates.

Device math per (g, f) pair, with F on partitions and G on the free dim:
  m1 = ab.p, m2 = ac.p, m3 = a.p           (PE matmuls, K=3)
  d1 = m1 - ab.a, d2 = m2 - ac.a
  d3 = d1 - |ab|^2, d4 = d2 - ab.ac, d5 = d1 - ab.ac, d6 = d2 - |ac|^2
  vc = d1 d4 - d3 d2, vb = d5 d2 - d1 d6, va = d3 d6 - d5 d4
  interior (v,w) = (vb, vc)/(va+vb+vc), then Ericson region overrides
  (edge bc / ca / ab, vertex a/b/c) via predicated copies; the three edge
  divisions collapse to per-face reciprocal multiplies because
  d4-d3+d5-d6 = |bc|^2, d2-d6 = |ac|^2, d1-d3 = |ab|^2 are per-face.
  score = dist^2 - |p|^2 = (|a|^2 - 2 m3) + v(v|ab|^2 - 2 d1)
          + w(w|ac|^2 - 2 d2) + 2(ab.ac)vw
  running (min, argmin) over face tiles, final cross-partition argmin via
  PE transpose + free-dim reduce (exact first-occurrence tie-breaking).
"""

import numpy as np

B, G, V, F = 2, 1024, 6890, 13776
NCORES = 8
FC = F // NCORES            # 1722 faces per core
FTILES = 14                 # ceil(1722/128)
FPAD = FTILES * 128         # 1792
GCHUNK = 512
NCONST = 28
OFF_FAB = 0
OFF_FAC = OFF_FAB + B * FPAD
OFF_FA6 = OFF_FAC + B * FPAD          # rows (-ax,-ay,-az,1,1,1)
OFF_PTS = OFF_FA6 + B * FPAD
OFF_P6 = OFF_PTS + B * G              # 3 coord blocks, rows: e_c at c, p_c at 3+c
OFF_FCO = OFF_P6 + 3 * B * G
OFF_ID = OFF_FCO + B * FTILES * NCONST
OFF_ONES = OFF_ID + 128
OFF_ZEROS = OFF_ONES + GCHUNK
OFF_INF = OFF_ZEROS + GCHUNK
WBLOB = OFF_INF + 128
PAD_SCORE = np.float32(4e37)
INF = np.float32(3e38)
EPS_DIV = np.float32(1e-24)

_CACHE = {}


def _build_bass():
    import concourse.bass as bass
    import concourse.bacc as bacc
    import concourse.mybir as mybir
    from concourse.tile import TileContext

    dt = mybir.dt.float32
    dtm = mybir.dt.uint8
    Alu = mybir.AluOpType
    Act = mybir.ActivationFunctionType
    Ax = mybir.AxisListType

    nc = bacc.Bacc()

    blob_d = nc.declare_dram_parameter("blob", [128, WBLOB], dt, isOutput=False)
    oidx_d = nc.declare_dram_parameter("out_idx", [B, G], dt, isOutput=True)
    oval_d = nc.declare_dram_parameter("out_val", [B, G], dt, isOutput=True)

    Vv = nc.vector
    Gg = nc.gpsimd
    Ss = nc.scalar
    Tt = nc.tensor
    Sy = nc.sync

    with TileContext(nc) as tc:
        with (
            tc.tile_pool(name="cpool", bufs=1) as cpool,
            tc.tile_pool(name="work", bufs=1) as work,
            tc.tile_pool(name="acc", bufs=2) as acc,
            tc.tile_pool(name="mm", bufs=2, space="PSUM") as mm,
            tc.tile_pool(name="outp", bufs=4) as outp,
        ):
            # persistent data: one blob, one DMA, one semaphore
            blob_s = cpool.tile([128, WBLOB], dt, name="blob_s")
            Sy.dma_start(blob_s[:], blob_d[:])
            ones_s = blob_s[:, OFF_ONES:OFF_ONES + GCHUNK]
            zeros_s = blob_s[:, OFF_ZEROS:OFF_ZEROS + GCHUNK]
            inf_s = blob_s[:, OFF_INF:OFF_INF + 128]
            fab_s = blob_s[0:3, OFF_FAB:OFF_FAB + B * FPAD]
            fac_s = blob_s[0:3, OFF_FAC:OFF_FAC + B * FPAD]
            fa6_s = blob_s[0:6, OFF_FA6:OFF_FA6 + B * FPAD]
            pts_s = blob_s[0:3, OFF_PTS:OFF_PTS + B * G]
            p6_s = blob_s[0:6, OFF_P6:OFF_P6 + 3 * B * G]
            ident_s = blob_s[:, OFF_ID:OFF_ID + 128]

            def C(b, ft, j):
                col = OFF_FCO + b * FTILES * NCONST + ft * NCONST + j
                return blob_s[:, col:col + 1]

            for b in range(B):
                for gc in range(G // GCHUNK):
                    g0 = b * G + gc * GCHUNK
                    P = pts_s[:, g0:g0 + GCHUNK]
                    best = acc.tile([128, GCHUNK], dt, name="best")
                    bidx = acc.tile([128, GCHUNK], dt, name="bidx")
                    Vv.memset(best[:], INF)
                    Vv.memset(bidx[:], 0.0)
                    for ft in range(FTILES):
                        f0 = b * FPAD + ft * 128
                        sh = [128, GCHUNK]
                        m1 = mm.tile(sh, dt, name="m1")
                        m2 = mm.tile(sh, dt, name="m2")
                        Tt.matmul(m1[:], fab_s[:, f0:f0 + 128], P, start=True, stop=True)
                        Tt.matmul(m2[:], fac_s[:, f0:f0 + 128], P, start=True, stop=True)
                        aps = []
                        for ci in range(3):
                            apc = mm.tile(sh, dt, name=f"ap{ci}", bufs=1)
                            Tt.matmul(apc[:],
                                      fa6_s[:, f0:f0 + 128],
                                      p6_s[:, ci * B * G + g0:ci * B * G + g0 + GCHUNK],
                                      start=True, stop=True)
                            aps.append(apc)
                        # consts: 0 kab, 1 kac, 2 naa, 3 nab, 4 ncc, 5 naa2,
                        # 6 nab2, 7 fidx, 8 -naa, 9 -nab, 10 -ncc, 11 naa-nab,
                        # 12 nab-ncc, 13 rbc, 14 rcc, 15 raa
                        d1 = work.tile(sh, dt, name="d1")
                        d2 = work.tile(sh, dt, name="d2")
                        Ss.activation(d1[:], m1[:], Act.Identity, bias=C(b, ft, 20))
                        Ss.activation(d2[:], m2[:], Act.Identity, bias=C(b, ft, 21))
                        d3 = work.tile(sh, dt, name="d3")
                        d4 = work.tile(sh, dt, name="d4")
                        d5 = work.tile(sh, dt, name="d5")
                        d6 = work.tile(sh, dt, name="d6")
                        Ss.activation(d3[:], d1[:], Act.Identity, bias=C(b, ft, 8))
                        Ss.activation(d4[:], d2[:], Act.Identity, bias=C(b, ft, 9))
                        Ss.activation(d5[:], d1[:], Act.Identity, bias=C(b, ft, 9))
                        Ss.activation(d6[:], d2[:], Act.Identity, bias=C(b, ft, 10))
                        dd = work.tile(sh, dt, name="dd")
                        Vv.tensor_tensor(dd[:], d1[:], d2[:], Alu.subtract)
                        # vc = naa*d2 - nab*d1, vb = ncc*d1 - nab*d2 (exact
                        # linear forms); va+vb+vc = |ab x ac|^2 = DEN per-face
                        t1 = work.tile(sh, dt, name="t1")
                        t2 = work.tile(sh, dt, name="t2")
                        vc = work.tile(sh, dt, name="vc")
                        vb = work.tile(sh, dt, name="vb")
                        van = work.tile(sh, dt, name="van")
                        Ss.activation(t1[:], d1[:], Act.Identity,
                                      bias=0.0, scale=C(b, ft, 23))
                        Vv.scalar_tensor_tensor(vc[:], d2[:], C(b, ft, 2), t1[:],
                                                Alu.mult, Alu.subtract)
                        Ss.activation(t2[:], d2[:], Act.Identity,
                                      bias=0.0, scale=C(b, ft, 23))
                        Vv.scalar_tensor_tensor(vb[:], d1[:], C(b, ft, 4), t2[:],
                                                Alu.mult, Alu.subtract)
                        Vv.tensor_tensor(t1[:], vb[:], vc[:], Alu.add)
                        Ss.activation(van[:], t1[:], Act.Identity, bias=C(b, ft, 24))
                        vv = work.tile(sh, dt, name="vv")
                        ww = work.tile(sh, dt, name="ww")
                        Ss.activation(vv[:], vb[:], Act.Identity,
                                      bias=0.0, scale=C(b, ft, 19))
                        Ss.activation(ww[:], vc[:], Act.Identity,
                                      bias=0.0, scale=C(b, ft, 19))
                        # edge bc: t_bc = (d4-d3)*rbc = (naamnab - dd)*rbc
                        u_bc = work.tile(sh, dt, name="u_bc")
                        t_bc = work.tile(sh, dt, name="t_bc")
                        omt_bc = work.tile(sh, dt, name="omt_bc")
                        Ss.activation(u_bc[:], dd[:], Act.Identity,
                                      bias=C(b, ft, 11), scale=-1.0)
                        Ss.activation(t_bc[:], u_bc[:], Act.Identity,
                                      bias=0.0, scale=C(b, ft, 13))
                        Ss.activation(omt_bc[:], t_bc[:], Act.Copy,
                                      bias=1.0, scale=-1.0)
                        cc = work.tile(sh, dt, name="cc")
                        cd = work.tile(sh, dt, name="cd")
                        msk = work.tile(sh, dtm, name="msk")
                        Vv.tensor_scalar(cc[:], dd[:], C(b, ft, 12), None, Alu.is_ge)
                        Vv.scalar_tensor_tensor(cd[:], dd[:], C(b, ft, 11), cc[:],
                                                Alu.is_le, Alu.mult)
                        Vv.scalar_tensor_tensor(msk[:], van[:], 0.0, cd[:],
                                                Alu.is_ge, Alu.mult)
                        Vv.copy_predicated(vv[:], msk[:], omt_bc[:])
                        Vv.copy_predicated(ww[:], msk[:], t_bc[:])
                        # edge ca: t_ac = d2*rcc ; mask (vb<=0)&(d2>=0)&(d2<=ncc)
                        t_ac = work.tile(sh, dt, name="t_ac")
                        Ss.activation(t_ac[:], d2[:], Act.Identity,
                                      bias=0.0, scale=C(b, ft, 14))
                        Vv.tensor_scalar(cc[:], d2[:], 0.0, None, Alu.is_ge)
                        Vv.scalar_tensor_tensor(cd[:], d2[:], C(b, ft, 4), cc[:],
                                                Alu.is_le, Alu.mult)
                        Vv.scalar_tensor_tensor(msk[:], vb[:], 0.0, cd[:],
                                                Alu.is_le, Alu.mult)
                        Vv.copy_predicated(vv[:], msk[:], zeros_s)
                        Vv.copy_predicated(ww[:], msk[:], t_ac[:])
                        # vertex c: (d6>=0)&(d5<=d6) -> (d2>=ncc)&(dd<=nabmncc)
                        Vv.tensor_scalar(cc[:], dd[:], C(b, ft, 12), None, Alu.is_le)
                        Vv.scalar_tensor_tensor(msk[:], d2[:], C(b, ft, 4), cc[:],
                                                Alu.is_ge, Alu.mult)
                        Vv.copy_predicated(vv[:], msk[:], zeros_s)
                        Vv.copy_predicated(ww[:], msk[:], ones_s)
                        # edge ab: t_ab = d1*raa ; mask (vc<=0)&(d1>=0)&(d1<=naa)
                        t_ab = work.tile(sh, dt, name="t_ab")
                        Ss.activation(t_ab[:], d1[:], Act.Identity,
                                      bias=0.0, scale=C(b, ft, 15))
                        Vv.tensor_scalar(cc[:], d1[:], 0.0, None, Alu.is_ge)
                        Vv.scalar_tensor_tensor(cd[:], d1[:], C(b, ft, 2), cc[:],
                                                Alu.is_le, Alu.mult)
                        Vv.scalar_tensor_tensor(msk[:], vc[:], 0.0, cd[:],
                                                Alu.is_le, Alu.mult)
                        Vv.copy_predicated(vv[:], msk[:], t_ab[:])
                        Vv.copy_predicated(ww[:], msk[:], zeros_s)
                        # vertex b: (d3>=0)&(d4<=d3) -> (d1>=naa)&(dd>=naamnab)
                        Vv.tensor_scalar(cc[:], d1[:], C(b, ft, 2), None, Alu.is_ge)
                        Vv.scalar_tensor_tensor(msk[:], dd[:], C(b, ft, 11), cc[:],
                                                Alu.is_ge, Alu.mult)
                        Vv.copy_predicated(vv[:], msk[:], ones_s)
                        Vv.copy_predicated(ww[:], msk[:], zeros_s)
                        # vertex a: (d1<=0)&(d2<=0)
                        Vv.tensor_scalar(cc[:], d2[:], 0.0, None, Alu.is_le)
                        Vv.scalar_tensor_tensor(msk[:], d1[:], 0.0, cc[:],
                                                Alu.is_le, Alu.mult)
                        Vv.copy_predicated(vv[:], msk[:], zeros_s)
                        Vv.copy_predicated(ww[:], msk[:], zeros_s)
                        # score: dist2 = sum_c (ap_c - v*ab_c - w*ac_c)^2,
                        # same differencing as the reference (consts 3,5,6 =
                        # -abx/-aby/-abz; 16,17,18 = -acx/-acy/-acz)
                        AB_COLS = (3, 5, 6)
                        AC_COLS = (16, 17, 18)
                        tq = []
                        for ci in range(3):
                            uc = work.tile(sh, dt, name=f"u{ci}")
                            tc_ = work.tile(sh, dt, name=f"t{ci}")
                            qc = work.tile(sh, dt, name=f"q{ci}")
                            Vv.scalar_tensor_tensor(uc[:], vv[:], C(b, ft, AB_COLS[ci]),
                                                    aps[ci][:], Alu.mult, Alu.add)
                            Vv.scalar_tensor_tensor(tc_[:], ww[:], C(b, ft, AC_COLS[ci]),
                                                    uc[:], Alu.mult, Alu.add)
                            Ss.activation(qc[:], tc_[:], Act.Square)
                            tq.append(qc)
                        s12 = work.tile(sh, dt, name="s12")
                        sc = work.tile(sh, dt, name="sc")
                        Vv.tensor_tensor(s12[:], tq[0][:], tq[1][:], Alu.add)
                        Vv.tensor_tensor(sc[:], s12[:], tq[2][:], Alu.add)
                        # running min + argmin
                        fidxt = work.tile(sh, dt, name="fidxt")
                        Ss.activation(fidxt[:], ones_s, Act.Copy,
                                      bias=0.0, scale=C(b, ft, 7))
                        Vv.tensor_tensor(msk[:], sc[:], best[:], Alu.is_lt)
                        Vv.copy_predicated(bidx[:], msk[:], fidxt[:])
                        Vv.tensor_tensor(best[:], best[:], sc[:], Alu.min)
                    # cross-partition argmin for this (b, gchunk)
                    for j in range(GCHUNK // 128):
                        bT = mm.tile([128, 128], dt, name="m1")
                        iT = mm.tile([128, 128], dt, name="m2")
                        Tt.transpose(bT[:], best[:, j * 128:(j + 1) * 128], ident_s)
                        Tt.transpose(iT[:], bidx[:, j * 128:(j + 1) * 128], ident_s)
                        minv = outp.tile([128, 1], dt, name="minv")
                        Vv.tensor_reduce(minv[:], bT[:], op=Alu.min, axis=Ax.X)
                        eqm = outp.tile([128, 128], dtm, name="eqm")
                        Vv.tensor_scalar(eqm[:], bT[:], minv[:], None, Alu.is_equal)
                        cand = outp.tile([128, 128], dt, name="cand")
                        Vv.tensor_copy(cand[:], inf_s)
                        Vv.copy_predicated(cand[:], eqm[:], iT[:])
                        idxv = outp.tile([128, 1], dt, name="idxv")
                        Vv.tensor_reduce(idxv[:], cand[:], op=Alu.min, axis=Ax.X)
                        gg0 = gc * GCHUNK + j * 128
                        Sy.dma_start(oidx_d[b, gg0:gg0 + 128], idxv[:, 0])
                        Sy.dma_start(oval_d[b, gg0:gg0 + 128], minv[:, 0])
    nc.finalize()
    return nc


def _get_nc():
    if "nc" not in _CACHE:
        _CACHE["nc"] = _build_bass()
    return _CACHE["nc"]


def _face_data(bv, faces):
    """Per-batch face vectors and constants, f32. Returns dict of arrays."""
    f32 = np.float32
    fverts = bv[faces]                    # [F,3,3]
    a = fverts[:, 0].astype(f32)
    ab = (fverts[:, 1] - fverts[:, 0]).astype(f32)
    ac = (fverts[:, 2] - fverts[:, 0]).astype(f32)
    naa = np.sum(ab * ab, -1, dtype=f32)
    nab = np.sum(ab * ac, -1, dtype=f32)
    ncc = np.sum(ac * ac, -1, dtype=f32)
    kab = np.sum(ab * a, -1, dtype=f32)
    kac = np.sum(ac * a, -1, dtype=f32)
    naa2 = np.sum(a * a, -1, dtype=f32)
    nbc = naa + ncc - f32(2.0) * nab      # |b-c|^2
    rbc = nbc / (nbc * nbc + EPS_DIV)
    den = naa * ncc - nab * nab                # |ab x ac|^2 = va+vb+vc
    rden = den / (den * den + EPS_DIV)
    rcc = ncc / (ncc * ncc + EPS_DIV)
    raa = naa / (naa * naa + EPS_DIV)
    return dict(a=a, ab=ab, ac=ac, naa=naa, nab=nab, ncc=ncc, kab=kab,
                kac=kac, naa2=naa2, rbc=rbc, rcc=rcc, raa=raa,
                den=den, rden=rden)


def _core_inputs(batch_garment_verts, batch_body_verts, body_faces):
    """Build the 8 per-core input maps."""
    f32 = np.float32
    pts = np.ascontiguousarray(
        batch_garment_verts.transpose(0, 2, 1)).astype(f32)   # [B,3,G]
    ident = np.eye(128, dtype=f32)
    fd = [_face_data(batch_body_verts[b], body_faces) for b in range(B)]
    in_maps = []
    for c in range(NCORES):
        sl = slice(c * FC, (c + 1) * FC)
        blob = np.zeros((128, WBLOB), f32)
        fab = np.zeros((B, 3, FPAD), f32)
        fac = np.zeros((B, 3, FPAD), f32)
        fa = np.zeros((B, 3, FPAD), f32)
        fco = np.zeros((B, FPAD, NCONST), f32)
        for b in range(B):
            d = fd[b]
            fab[b, :, :FC] = d["ab"][sl].T
            fac[b, :, :FC] = d["ac"][sl].T
            fa[b, :, :FC] = d["a"][sl].T
            zc = np.zeros(FC, f32)
            cols = [d["kab"][sl], d["kac"][sl], d["naa"][sl],
                    -d["ab"][sl, 0], d["ncc"][sl], -d["ab"][sl, 1],
                    -d["ab"][sl, 2],
                    np.arange(FC, dtype=f32), -d["naa"][sl], -d["nab"][sl],
                    -d["ncc"][sl], d["naa"][sl] - d["nab"][sl],
                    d["nab"][sl] - d["ncc"][sl], d["rbc"][sl],
                    d["rcc"][sl], d["raa"][sl],
                    -d["ac"][sl, 0], -d["ac"][sl, 1], -d["ac"][sl, 2],
                    d["rden"][sl], -d["kab"][sl], -d["kac"][sl],
                    d["den"][sl], d["nab"][sl], -d["den"][sl],
                    zc, zc, zc]
            fco[b, :FC, :] = np.stack(cols, axis=1)
        # [B, FPAD, NCONST] -> [B, 128, FTILES*NCONST]
        fco_t = fco.reshape(B, FTILES, 128, NCONST).transpose(0, 2, 1, 3)
        fco_t = np.ascontiguousarray(fco_t).reshape(B, 128, FTILES * NCONST)
        blob[0:3, OFF_FAB:OFF_FAB + B * FPAD] = fab.transpose(1, 0, 2).reshape(3, -1)
        blob[0:3, OFF_FAC:OFF_FAC + B * FPAD] = fac.transpose(1, 0, 2).reshape(3, -1)
        fa6 = np.zeros((B, 6, FPAD), f32)
        for b in range(B):
            for ci in range(3):
                fa6[b, ci, :FC] = -fd[b]["a"][sl][:, ci]
                fa6[b, ci, FC:] = f32(1e18)          # pad faces: huge dist2
                fa6[b, 3 + ci, :] = 1.0
        blob[0:6, OFF_FA6:OFF_FA6 + B * FPAD] = fa6.transpose(1, 0, 2).reshape(6, -1)
        blob[0:3, OFF_PTS:OFF_PTS + B * G] = pts.transpose(1, 0, 2).reshape(3, -1)
        p6 = np.zeros((3, 6, B * G), f32)
        for ci in range(3):
            p6[ci, ci, :] = 1.0
            p6[ci, 3 + ci, :] = pts[:, ci, :].reshape(-1)
        blob[0:6, OFF_P6:OFF_P6 + 3 * B * G] = \
            p6.transpose(1, 0, 2).reshape(6, -1)
        blob[:, OFF_FCO:OFF_FCO + B * FTILES * NCONST] = \
            fco_t.transpose(1, 0, 2).reshape(128, -1)
        blob[:, OFF_ID:OFF_ID + 128] = ident
        blob[:, OFF_ONES:OFF_ONES + GCHUNK] = 1.0
        blob[:, OFF_INF:OFF_INF + 128] = INF
        in_maps.append({"blob": blob})
    return in_maps


def _ericson(g_verts, b_verts, faces, tri):
    """Reference Ericson for the chosen face of each point: (v, w, part, npt)."""
    f32 = np.float32

    def safe(x):
        return np.where(np.abs(x) < 1e-12, f32(1e-12), x).astype(f32)

    fverts = b_verts[faces[tri]]
    a, bb, cc = fverts[:, 0], fverts[:, 1], fverts[:, 2]
    q = g_verts
    ab = bb - a; ac = cc - a
    ap = q - a
    d1 = np.sum(ab * ap, -1); d2 = np.sum(ac * ap, -1)
    bp = q - bb
    d3 = np.sum(ab * bp, -1); d4 = np.sum(ac * bp, -1)
    cp = q - cc
    d5 = np.sum(ab * cp, -1); d6 = np.sum(ac * cp, -1)
    vc = d1 * d4 - d3 * d2
    vb = d5 * d2 - d1 * d6
    va = d3 * d6 - d5 * d4
    denom = safe(va + vb + vc)
    v, w = (vb / denom).astype(f32), (vc / denom).astype(f32)
    part = np.zeros(v.shape, np.int32)
    t_bc = ((d4 - d3) / safe((d4 - d3) + (d5 - d6))).astype(f32)
    m = (va <= 0) & (d4 - d3 >= 0) & (d5 - d6 >= 0)
    v = np.where(m, 1.0 - t_bc, v).astype(f32)
    w = np.where(m, t_bc, w).astype(f32)
    part = np.where(m, 2, part)
    t_ac = (d2 / safe(d2 - d6)).astype(f32)
    m = (vb <= 0) & (d2 >= 0) & (d6 <= 0)
    v = np.where(m, 0.0, v).astype(f32)
    w = np.where(m, t_ac, w).astype(f32)
    part = np.where(m, 3, part)
    m = (d6 >= 0) & (d5 <= d6)
    v = np.where(m, 0.0, v).astype(f32)
    w = np.where(m, 1.0, w).astype(f32)
    part = np.where(m, 6, part)
    t_ab = (d1 / safe(d1 - d3)).astype(f32)
    m = (vc <= 0) & (d1 >= 0) & (d3 <= 0)
    v = np.where(m, t_ab, v).astype(f32)
    w = np.where(m, 0.0, w).astype(f32)
    part = np.where(m, 1, part)
    m = (d3 >= 0) & (d4 <= d3)
    v = np.where(m, 1.0, v).astype(f32)
    w = np.where(m, 0.0, w).astype(f32)
    part = np.where(m, 5, part)
    m = (d1 <= 0) & (d2 <= 0)
    v = np.where(m, 0.0, v).astype(f32)
    w = np.where(m, 0.0, w).astype(f32)
    part = np.where(m, 4, part)
    npt = a + v[:, None] * ab + w[:, None] * ac
    return v, w, part, npt


def _dist2_ref(g_verts, b_verts, faces, tri):
    _, _, _, npt = _ericson(g_verts, b_verts, faces, tri)
    return np.sum((g_verts - npt) ** 2, -1).astype(np.float32)


def _host_finish(g_verts, b_verts, faces, tri):
    """Exact reference finish for the winning face of each garment point."""
    f32 = np.float32
    EPS = f32(1e-10)

    def safe(x):
        return np.where(np.abs(x) < 1e-12, f32(1e-12), x).astype(f32)

    fverts = b_verts[faces]
    a_, b_, c_ = fverts[:, 0], fverts[:, 1], fverts[:, 2]
    fn_raw = np.cross(b_ - a_, c_ - a_).astype(f32)
    vn = np.zeros_like(b_verts)
    for k in range(3):
        np.add.at(vn, faces[:, k], fn_raw)
    vn = vn / (np.linalg.norm(vn, axis=-1, keepdims=True).astype(f32) + EPS)
    fn = fn_raw / (np.linalg.norm(fn_raw, axis=-1, keepdims=True).astype(f32) + EPS)

    a = a_[tri]; bb = b_[tri]; cc = c_[tri]
    q = g_verts
    ab = bb - a; ac = cc - a
    ap = q - a
    d1 = np.sum(ab * ap, -1); d2 = np.sum(ac * ap, -1)
    bp = q - bb
    d3 = np.sum(ab * bp, -1); d4 = np.sum(ac * bp, -1)
    cp = q - cc
    d5 = np.sum(ab * cp, -1); d6 = np.sum(ac * cp, -1)
    vc = d1 * d4 - d3 * d2
    vb = d5 * d2 - d1 * d6
    va = d3 * d6 - d5 * d4
    denom = safe(va + vb + vc)
    v, w = (vb / denom).astype(f32), (vc / denom).astype(f32)
    part = np.zeros(v.shape, np.int32)
    t_bc = ((d4 - d3) / safe((d4 - d3) + (d5 - d6))).astype(f32)
    m = (va <= 0) & (d4 - d3 >= 0) & (d5 - d6 >= 0)
    v = np.where(m, 1.0 - t_bc, v).astype(f32)
    w = np.where(m, t_bc, w).astype(f32)
    part = np.where(m, 2, part)
    t_ac = (d2 / safe(d2 - d6)).astype(f32)
    m = (vb <= 0) & (d2 >= 0) & (d6 <= 0)
    v = np.where(m, 0.0, v).astype(f32)
    w = np.where(m, t_ac, w).astype(f32)
    part = np.where(m, 3, part)
    m = (d6 >= 0) & (d5 <= d6)
    v = np.where(m, 0.0, v).astype(f32)
    w = np.where(m, 1.0, w).astype(f32)
    part = np.where(m, 6, part)
    t_ab = (d1 / safe(d1 - d3)).astype(f32)
    m = (vc <= 0) & (d1 >= 0) & (d3 <= 0)
    v = np.where(m, t_ab, v).astype(f32)
    w = np.where(m, 0.0, w).astype(f32)
    part = np.where(m, 1, part)
    m = (d3 >= 0) & (d4 <= d3)
    v = np.where(m, 1.0, v).astype(f32)
    w = np.where(m, 0.0, w).astype(f32)
    part = np.where(m, 5, part)
    m = (d1 <= 0) & (d2 <= 0)
    v = np.where(m, 0.0, v).astype(f32)
    w = np.where(m, 0.0, w).astype(f32)
    part = np.where(m, 4, part)
    npt = a + v[:, None] * ab + w[:, None] * ac

    fidx = faces[tri]
    gar = np.arange(len(tri))
    take = lambda col: vn[fidx[gar, col]]
    n_face = fn[tri]
    n_vert = take(np.clip(part - 4, 0, 2))
    n_edge = take(np.clip(part - 1, 0, 2)) + take(np.mod(part, 3))
    n = np.where((part == 0)[:, None], n_face,
                 np.where((part > 3)[:, None], n_vert, n_edge)).astype(f32)
    n = n / (np.linalg.norm(n, axis=-1, keepdims=True).astype(f32) + EPS)
    return np.sum((g_verts - npt) * n, axis=1).astype(f32)


def kernel(batch_garment_verts, batch_body_verts, body_faces, _profile=None):
    from concourse.bass_utils import run_bass_kernel_spmd

    batch_garment_verts = np.asarray(batch_garment_verts, dtype=np.float32)
    batch_body_verts = np.asarray(batch_body_verts, dtype=np.float32)
    body_faces = np.asarray(body_faces)

    nc = _get_nc()
    in_maps = _core_inputs(batch_garment_verts, batch_body_verts, body_faces)
    kwargs = dict(_profile) if _profile else {}
    res = run_bass_kernel_spmd(nc, in_maps, list(range(NCORES)), **kwargs)
    if _profile is not None:
        _CACHE["last_results"] = res

    vals = np.stack([r["out_val"] for r in res.results])   # [8, B, G]
    idxs = np.stack([r["out_idx"] for r in res.results])
    # exact re-rank of the 8 per-core candidates with reference-style dist2
    cand = (idxs + np.arange(NCORES, dtype=np.float32)[:, None, None] * FC
            ).astype(np.int64)                             # [8, B, G] global idx
    tri = np.empty((B, G), np.int64)
    for b in range(B):
        d2c = np.stack([
            _dist2_ref(batch_garment_verts[b], batch_body_verts[b],
                       body_faces, cand[c, b]) for c in range(NCORES)])
        bestd = d2c[0].copy()
        besti = cand[0, b].copy()
        for c in range(1, NCORES):
            upd = (d2c[c] < bestd) | ((d2c[c] == bestd) & (cand[c, b] < besti))
            bestd = np.where(upd, d2c[c], bestd)
            besti = np.where(upd, cand[c, b], besti)
        tri[b] = besti
    _CACHE["tri"] = tri
    _CACHE["vals"] = vals
    _CACHE["idxs"] = idxs

    out = np.empty((B, G), np.float32)
    for b in range(B):
        out[b] = _host_finish(batch_garment_verts[b], batch_body_verts[b],
                              body_faces, tri[b])
    return out



# revision 9
# speedup vs baseline: 2.4270x; 2.4270x over previous
"""Accurate SDF (garment-to-body signed distance) on 8 Trainium2 cores — v2.

Faces sharded 8 ways (1722/core, padded to 14*128); every core scores all
B*G garment points against its faces and returns per-PSUM-partition running
minima [B, 128, G] (no on-device argmin). Host takes the top-M partitions
per point by device score, exactly re-ranks their 14 faces each in fp64,
and finishes (region code, normals, sign) with the reference formulas.

Device math per (face f, point g), with faces on partitions and g on the
free dim (moving rows P5 = [px, py, pz, 1, |p|^2]):
  edge e (seg anchor v_e, unit dir u_e, length L_e):
    U_e = u_e.(p - v_e)                (fp32 matmul)
    T_e = clamp(U_e, 0, L_e)           (relu on Act + min on DVE/Pool)
    w_e = T_e*(2U_e - T_e)             so d2_e = |p - v_e|^2 - w_e
  A    = |p - a|^2                     (fp32 matmul, |p|^2 row)
  A_b  = A + D',  D' = -2 L_ab U_ab + L_ab^2   (Act scale/bias from U_ab)
  face: h = n^.(p - a)  (fp32 matmul), score h^2, masked by the sign of
    vb, vc, va = den - vb - vc (row-normalized fp32r matmuls) via a
    BIG*relu(-min(...)) penalty.
  sc = min(A - max(w_ab, w_ca), A_b - w_bc, h^2 + penalty)
  best[partition] = min over ft tiles  ->  DMA out per (b, gchunk).
"""

import numpy as np

B, G, V, F = 2, 1024, 6890, 13776
NCORES = 8
FC = F // NCORES            # 1722 faces per core
FTILES = 14                 # ceil(1722/128)
FPAD = FTILES * 128         # 1792
GCHUNK = 512
NMM5 = 5                    # fp32 matmuls: U_ab, U_ca, U_bc, A, h
NMM3 = 3                    # fp32r matmuls: vb, vc, va
NCST = 5                    # ptr consts: L_ab, L_ca, L_bc, -2L_ab, L_ab^2
W5COLS = B * FTILES * NMM5 * 128
W3COLS = B * FTILES * NMM3 * 128
CSTCOLS = B * FTILES * NCST
BIG = np.float32(1e6)
INF = np.float32(3e38)
TOPM = 16                   # host: partitions re-ranked exactly per point

_CACHE = {}


def _build_bass():
    import concourse.bass as bass
    import concourse.bacc as bacc
    import concourse.mybir as mybir
    from concourse.tile import TileContext

    dt = mybir.dt.float32
    dtr = mybir.dt.float32r
    Alu = mybir.AluOpType
    Act = mybir.ActivationFunctionType

    nc = bacc.Bacc()

    w5_d = nc.declare_dram_parameter("w5", [NMM5, W5COLS], dt, isOutput=False)
    w3_d = nc.declare_dram_parameter("w3r", [NMM5, W3COLS], dtr, isOutput=False)
    p5_d = nc.declare_dram_parameter("p5", [NMM5, B * G], dt, isOutput=False)
    p5r_d = nc.declare_dram_parameter("p5r", [NMM5, B * G], dtr, isOutput=False)
    cst_d = nc.declare_dram_parameter("cst", [128, CSTCOLS], dt, isOutput=False)
    oval_d = nc.declare_dram_parameter("out_val", [B, 128, G], dt, isOutput=True)

    Vv = nc.vector
    Gg = nc.gpsimd
    Ss = nc.scalar
    Tt = nc.tensor
    Sy = nc.sync

    with TileContext(nc) as tc:
        with (
            tc.tile_pool(name="cpool", bufs=1) as cpool,
            tc.tile_pool(name="work", bufs=1) as work,
            tc.tile_pool(name="acc", bufs=2) as acc,
            tc.tile_pool(name="mm", bufs=1, space="PSUM") as mm,
        ):
            w5_s = cpool.tile([NMM5, W5COLS], dt, name="w5_s")
            w3_s = cpool.tile([NMM5, W3COLS], dtr, name="w3_s")
            p5_s = cpool.tile([NMM5, B * G], dt, name="p5_s")
            p5r_s = cpool.tile([NMM5, B * G], dtr, name="p5r_s")
            cst_s = cpool.tile([128, CSTCOLS], dt, name="cst_s")
            Sy.dma_start(w5_s[:], w5_d[:])
            Sy.dma_start(w3_s[:], w3_d[:])
            Sy.dma_start(p5_s[:], p5_d[:])
            Sy.dma_start(p5r_s[:], p5r_d[:])
            Sy.dma_start(cst_s[:], cst_d[:])

            def W5(b, ft, m):
                c = ((b * FTILES + ft) * NMM5 + m) * 128
                return w5_s[:, c:c + 128]

            def W3(b, ft, m):
                c = ((b * FTILES + ft) * NMM3 + m) * 128
                return w3_s[:, c:c + 128]

            def CST(b, ft, j):
                c = (b * FTILES + ft) * NCST + j
                return cst_s[:, c:c + 1]

            sh = [128, GCHUNK]
            for b in range(B):
                for gc in range(G // GCHUNK):
                    g0 = b * G + gc * GCHUNK
                    P = p5_s[:, g0:g0 + GCHUNK]
                    Pr = p5r_s[:, g0:g0 + GCHUNK]
                    best = acc.tile(sh, dt, name="best")
                    Vv.memset(best[:], INF)
                    for ft in range(FTILES):
                        u_ab = mm.tile(sh, dt, name="u_ab")
                        u_ca = mm.tile(sh, dt, name="u_ca")
                        u_bc = mm.tile(sh, dt, name="u_bc")
                        am = mm.tile(sh, dt, name="am")
                        hm = mm.tile(sh, dt, name="hm")
                        vbm = mm.tile(sh, dt, name="vbm")
                        vcm = mm.tile(sh, dt, name="vcm")
                        vam = mm.tile(sh, dt, name="vam")
                        Tt.matmul(u_ab[:], W5(b, ft, 0), P, start=True, stop=True)
                        Tt.matmul(u_ca[:], W5(b, ft, 1), P, start=True, stop=True)
                        Tt.matmul(u_bc[:], W5(b, ft, 2), P, start=True, stop=True)
                        Tt.matmul(am[:], W5(b, ft, 3), P, start=True, stop=True)
                        Tt.matmul(hm[:], W5(b, ft, 4), P, start=True, stop=True)
                        Tt.matmul(vbm[:], W3(b, ft, 0), Pr, start=True, stop=True)
                        Tt.matmul(vcm[:], W3(b, ft, 1), Pr, start=True, stop=True)
                        Tt.matmul(vam[:], W3(b, ft, 2), Pr, start=True, stop=True)
                        # Act: drain psum fast
                        r1ab = work.tile(sh, dt, name="r1ab")
                        r1ca = work.tile(sh, dt, name="r1ca")
                        r1bc = work.tile(sh, dt, name="r1bc")
                        sf = work.tile(sh, dt, name="sf")
                        dp = work.tile(sh, dt, name="dp")
                        a_s = work.tile(sh, dt, name="a_s")
                        Ss.activation(r1ab[:], u_ab[:], Act.Relu)
                        Ss.activation(r1ca[:], u_ca[:], Act.Relu)
                        Ss.activation(r1bc[:], u_bc[:], Act.Relu)
                        Ss.activation(sf[:], hm[:], Act.Square)
                        Ss.activation(dp[:], u_ab[:], Act.Identity,
                                      bias=CST(b, ft, 4), scale=CST(b, ft, 3))
                        Ss.activation(a_s[:], am[:], Act.Identity)
                        # clamp T = min(relu(U), L)
                        t_ab = work.tile(sh, dt, name="t_ab")
                        t_ca = work.tile(sh, dt, name="t_ca")
                        t_bc = work.tile(sh, dt, name="t_bc")
                        Vv.tensor_scalar(t_ab[:], r1ab[:], CST(b, ft, 0), None, Alu.min)
                        Vv.tensor_scalar(t_ca[:], r1ca[:], CST(b, ft, 1), None, Alu.min)
                        Vv.tensor_scalar(t_bc[:], r1bc[:], CST(b, ft, 2), None, Alu.min)
                        # z = 2*relu(U) - T  (== 2U - T wherever T != 0)
                        z_ab = work.tile(sh, dt, name="z_ab")
                        z_ca = work.tile(sh, dt, name="z_ca")
                        z_bc = work.tile(sh, dt, name="z_bc")
                        Vv.scalar_tensor_tensor(z_ab[:], r1ab[:], 2.0, t_ab[:],
                                                Alu.mult, Alu.subtract)
                        Vv.scalar_tensor_tensor(z_ca[:], r1ca[:], 2.0, t_ca[:],
                                                Alu.mult, Alu.subtract)
                        Vv.scalar_tensor_tensor(z_bc[:], r1bc[:], 2.0, t_bc[:],
                                                Alu.mult, Alu.subtract)
                        w_ab = work.tile(sh, dt, name="w_ab")
                        w_ca = work.tile(sh, dt, name="w_ca")
                        w_bc = work.tile(sh, dt, name="w_bc")
                        Gg.tensor_tensor(w_ab[:], t_ab[:], z_ab[:], Alu.mult)
                        Gg.tensor_tensor(w_ca[:], t_ca[:], z_ca[:], Alu.mult)
                        Gg.tensor_tensor(w_bc[:], t_bc[:], z_bc[:], Alu.mult)
                        # face mask: penalty = BIG*relu(-min(vb,vc,va))
                        mn1 = work.tile(sh, dt, name="mn1")
                        mn2 = work.tile(sh, dt, name="mn2")
                        rneg = work.tile(sh, dt, name="rneg")
                        sfm = work.tile(sh, dt, name="sfm")
                        vc_s = work.tile(sh, dt, name="vc_s")
                        Ss.activation(vc_s[:], vcm[:], Act.Identity)
                        Vv.tensor_tensor(mn1[:], vbm[:], vc_s[:], Alu.min)
                        Vv.tensor_tensor(mn2[:], mn1[:], vam[:], Alu.min)
                        Ss.activation(rneg[:], mn2[:], Act.Relu, scale=-1.0)
                        Vv.scalar_tensor_tensor(sfm[:], rneg[:], float(BIG), sf[:],
                                                Alu.mult, Alu.add)
                        # combine
                        mx = work.tile(sh, dt, name="mx")
                        e1 = work.tile(sh, dt, name="e1")
                        tb2 = work.tile(sh, dt, name="tb2")
                        e2 = work.tile(sh, dt, name="e2")
                        em = work.tile(sh, dt, name="em")
                        sc = work.tile(sh, dt, name="sc")
                        Vv.tensor_tensor(mx[:], w_ab[:], w_ca[:], Alu.max)
                        Gg.tensor_tensor(e1[:], a_s[:], mx[:], Alu.subtract)
                        Gg.tensor_tensor(tb2[:], dp[:], w_bc[:], Alu.subtract)
                        Gg.tensor_tensor(e2[:], a_s[:], tb2[:], Alu.add)
                        Vv.tensor_tensor(em[:], e1[:], e2[:], Alu.min)
                        Vv.tensor_tensor(sc[:], em[:], sfm[:], Alu.min)
                        Vv.tensor_tensor(best[:], best[:], sc[:], Alu.min)
                    gg0 = gc * GCHUNK
                    Sy.dma_start(oval_d[b, :, gg0:gg0 + GCHUNK], best[:])
    nc.finalize()
    return nc


def _get_nc():
    if "nc" not in _CACHE:
        _CACHE["nc"] = _build_bass()
    return _CACHE["nc"]


def _round_fp32r(x):
    """Round fp32 -> fp32r container (11-bit mantissa, RNE)."""
    u = np.ascontiguousarray(x, np.float32).view(np.uint32)
    base = u & np.uint32(0xFFFFF000)
    low = u & np.uint32(0x00000FFF)
    half = np.uint32(0x800)
    lsb = (base >> np.uint32(12)) & np.uint32(1)
    up = (low > half) | ((low == half) & (lsb == 1))
    return np.where(up, base + np.uint32(0x1000), base).view(np.float32)


def _core_inputs(batch_garment_verts, batch_body_verts, body_faces):
    f32 = np.float32
    gv = batch_garment_verts.astype(f32)
    p5 = np.concatenate(
        [gv.transpose(0, 2, 1),                       # [B,3,G]
         np.ones((B, 1, G), f32),
         np.sum(gv * gv, -1, dtype=f32)[:, None, :]], 1
    ).reshape(B * NMM5, G)
    p5 = np.ascontiguousarray(
        p5.reshape(B, NMM5, G).transpose(1, 0, 2)).reshape(NMM5, B * G)
    p5r = _round_fp32r(p5)

    in_maps = []
    for c in range(NCORES):
        sl = slice(c * FC, (c + 1) * FC)
        w5 = np.zeros((NMM5, W5COLS), f32)
        w3 = np.zeros((NMM5, W3COLS), f32)
        cst = np.zeros((128, CSTCOLS), f32)
        for b in range(B):
            fv = batch_body_verts[b].astype(f32)[body_faces[sl]]  # [FC,3,3]
            a, bb, cc = fv[:, 0], fv[:, 1], fv[:, 2]
            n = np.cross((bb - a).astype(np.float64),
                         (cc - a).astype(np.float64))
            nn = np.linalg.norm(n, axis=1)
            ab, ac = (bb - a).astype(f32), (cc - a).astype(f32)
            naa = np.sum(ab * ab, -1, dtype=f32)
            nab = np.sum(ab * ac, -1, dtype=f32)
            ncc = np.sum(ac * ac, -1, dtype=f32)
            den = (naa * ncc - nab * nab).astype(f32)
            degen = (den < f32(1e-4)) | (nn < 1e-10)
            nh = np.where(degen[:, None], 0,
                          n / np.maximum(nn, 1e-30)[:, None]).astype(f32)

            def pads(rows):  # [FC,5] -> [5, FPAD] padded
                out = np.zeros((NMM5, FPAD), f32)
                out[:, :FC] = rows.T
                return out

            Ls = {}
            Ws = {}
            for nm, (ea, eb) in (("ab", (a, bb)), ("ca", (a, cc)),
                                 ("bc", (bb, cc))):
                ed = (eb - ea).astype(f32)
                L = np.linalg.norm(ed.astype(np.float64), axis=1).astype(f32)
                u = np.where(L[:, None] > 0,
                             ed / np.maximum(L, f32(1e-30))[:, None], 0)
                Ws[nm] = pads(np.concatenate(
                    [u, (-np.sum(u * ea, -1, dtype=f32))[:, None],
                     np.zeros((FC, 1), f32)], 1))
                Lp = np.zeros(FPAD, f32)
                Lp[:FC] = L
                Ls[nm] = Lp
            wa_rows = np.concatenate(
                [-2 * a, np.sum(a * a, -1, dtype=f32)[:, None],
                 np.ones((FC, 1), f32)], 1)
            WA = pads(wa_rows)
            WA[3, FC:] = f32(1e30)                       # pad faces: huge A
            Wh = pads(np.concatenate(
                [nh, (-np.sum(nh * a, -1, dtype=f32))[:, None],
                 np.zeros((FC, 1), f32)], 1))
            wvb = (ncc[:, None] * ab - nab[:, None] * ac).astype(f32)
            wvc = (naa[:, None] * ac - nab[:, None] * ab).astype(f32)
            cvb = (-np.sum(wvb * a, -1, dtype=f32))
            cvc = (-np.sum(wvc * a, -1, dtype=f32))
            wva = (-(wvb + wvc)).astype(f32)
            cva = (den - cvb - cvc).astype(f32)
            W3L = []
            for w_, c_ in ((wvb, cvb), (wvc, cvc), (wva, cva)):
                s = np.maximum(np.linalg.norm(w_, axis=1), f32(1e-30)).astype(f32)
                W3L.append(pads(np.concatenate(
                    [w_ / s[:, None], (c_ / s)[:, None],
                     np.zeros((FC, 1), f32)], 1)))
            Wvb, Wvc, Wva = W3L
            Wva[:, :FC][:, degen] = 0.0
            Wva[3, :FC][degen] = -1.0
            Wva[:, FC:] = 0.0
            Wva[3, FC:] = -1.0                           # pad faces: outside

            mm5 = [Ws["ab"], Ws["ca"], Ws["bc"], WA, Wh]
            mm3 = [Wvb, Wvc, Wva]
            for ft in range(FTILES):
                fsl = slice(ft * 128, (ft + 1) * 128)
                for m in range(NMM5):
                    c0 = ((b * FTILES + ft) * NMM5 + m) * 128
                    w5[:, c0:c0 + 128] = mm5[m][:, fsl]
                for m in range(NMM3):
                    c0 = ((b * FTILES + ft) * NMM3 + m) * 128
                    w3[:, c0:c0 + 128] = mm3[m][:, fsl]
                c0 = (b * FTILES + ft) * NCST
                cst[:, c0 + 0] = Ls["ab"][fsl]
                cst[:, c0 + 1] = Ls["ca"][fsl]
                cst[:, c0 + 2] = Ls["bc"][fsl]
                cst[:, c0 + 3] = -2 * Ls["ab"][fsl]
                cst[:, c0 + 4] = Ls["ab"][fsl] ** 2
        in_maps.append({"w5": w5, "w3r": _round_fp32r(w3),
                        "p5": p5, "p5r": p5r, "cst": cst})
    return in_maps


def _d2_exact64_cand(p, bverts, faces, cand):
    """Exact fp64 point-triangle dist^2 for candidate faces. cand [G,C]."""
    fv = bverts[faces[cand]].astype(np.float64)      # [G,C,3,3]
    a, b, c = fv[:, :, 0], fv[:, :, 1], fv[:, :, 2]
    q = p.astype(np.float64)[:, None, :]
    best = np.full(cand.shape, np.inf)
    for ea, eb in ((a, b), (b, c), (c, a)):
        ed = eb - ea
        L2 = np.sum(ed * ed, -1)
        pe = q - ea
        t = np.clip(np.sum(pe * ed, -1) / np.maximum(L2, 1e-300), 0, 1)
        d = pe - t[..., None] * ed
        best = np.minimum(best, np.sum(d * d, -1))
    ab, ac = b - a, c - a
    n = np.cross(ab, ac)
    naa = np.sum(ab * ab, -1); nab = np.sum(ab * ac, -1)
    ncc = np.sum(ac * ac, -1)
    den = naa * ncc - nab * nab
    pa = q - a
    d1 = np.sum(pa * ab, -1); d2_ = np.sum(pa * ac, -1)
    vb = ncc * d1 - nab * d2_; vc = naa * d2_ - nab * d1
    va = den - vb - vc
    inside = (vb >= 0) & (vc >= 0) & (va >= 0) & (den > 1e-300)
    hn = np.sum(pa * n, -1)
    h2 = hn * hn / np.maximum(den, 1e-300)
    return np.where(inside, np.minimum(best, h2), best)


def _host_finish(g_verts, b_verts, faces, tri):
    """Exact reference finish for the winning face of each garment point."""
    f32 = np.float32
    EPS = f32(1e-10)

    def safe(x):
        return np.where(np.abs(x) < 1e-12, f32(1e-12), x).astype(f32)

    fverts = b_verts[faces]
    a_, b_, c_ = fverts[:, 0], fverts[:, 1], fverts[:, 2]
    fn_raw = np.cross(b_ - a_, c_ - a_).astype(f32)
    vn = np.zeros_like(b_verts)
    for k in range(3):
        np.add.at(vn, faces[:, k], fn_raw)
    vn = vn / (np.linalg.norm(vn, axis=-1, keepdims=True).astype(f32) + EPS)
    fn = fn_raw / (np.linalg.norm(fn_raw, axis=-1, keepdims=True).astype(f32) + EPS)

    a = a_[tri]; bb = b_[tri]; cc = c_[tri]
    q = g_verts
    ab = bb - a; ac = cc - a
    ap = q - a
    d1 = np.sum(ab * ap, -1); d2 = np.sum(ac * ap, -1)
    bp = q - bb
    d3 = np.sum(ab * bp, -1); d4 = np.sum(ac * bp, -1)
    cp = q - cc
    d5 = np.sum(ab * cp, -1); d6 = np.sum(ac * cp, -1)
    vc = d1 * d4 - d3 * d2
    vb = d5 * d2 - d1 * d6
    va = d3 * d6 - d5 * d4
    denom = safe(va + vb + vc)
    v, w = (vb / denom).astype(f32), (vc / denom).astype(f32)
    part = np.zeros(v.shape, np.int32)
    t_bc = ((d4 - d3) / safe((d4 - d3) + (d5 - d6))).astype(f32)
    m = (va <= 0) & (d4 - d3 >= 0) & (d5 - d6 >= 0)
    v = np.where(m, 1.0 - t_bc, v).astype(f32)
    w = np.where(m, t_bc, w).astype(f32)
    part = np.where(m, 2, part)
    t_ac = (d2 / safe(d2 - d6)).astype(f32)
    m = (vb <= 0) & (d2 >= 0) & (d6 <= 0)
    v = np.where(m, 0.0, v).astype(f32)
    w = np.where(m, t_ac, w).astype(f32)
    part = np.where(m, 3, part)
    m = (d6 >= 0) & (d5 <= d6)
    v = np.where(m, 0.0, v).astype(f32)
    w = np.where(m, 1.0, w).astype(f32)
    part = np.where(m, 6, part)
    t_ab = (d1 / safe(d1 - d3)).astype(f32)
    m = (vc <= 0) & (d1 >= 0) & (d3 <= 0)
    v = np.where(m, t_ab, v).astype(f32)
    w = np.where(m, 0.0, w).astype(f32)
    part = np.where(m, 1, part)
    m = (d3 >= 0) & (d4 <= d3)
    v = np.where(m, 1.0, v).astype(f32)
    w = np.where(m, 0.0, w).astype(f32)
    part = np.where(m, 5, part)
    m = (d1 <= 0) & (d2 <= 0)
    v = np.where(m, 0.0, v).astype(f32)
    w = np.where(m, 0.0, w).astype(f32)
    part = np.where(m, 4, part)
    npt = a + v[:, None] * ab + w[:, None] * ac

    fidx = faces[tri]
    gar = np.arange(len(tri))
    take = lambda col: vn[fidx[gar, col]]
    n_face = fn[tri]
    n_vert = take(np.clip(part - 4, 0, 2))
    n_edge = take(np.clip(part - 1, 0, 2)) + take(np.mod(part, 3))
    n = np.where((part == 0)[:, None], n_face,
                 np.where((part > 3)[:, None], n_vert, n_edge)).astype(f32)
    n = n / (np.linalg.norm(n, axis=-1, keepdims=True).astype(f32) + EPS)
    return np.sum((g_verts - npt) * n, axis=1).astype(f32)


def kernel(batch_garment_verts, batch_body_verts, body_faces, _profile=None):
    from concourse.bass_utils import run_bass_kernel_spmd

    batch_garment_verts = np.asarray(batch_garment_verts, dtype=np.float32)
    batch_body_verts = np.asarray(batch_body_verts, dtype=np.float32)
    body_faces = np.asarray(body_faces)

    nc = _get_nc()
    in_maps = _core_inputs(batch_garment_verts, batch_body_verts, body_faces)
    kwargs = dict(_profile) if _profile else {}
    res = run_bass_kernel_spmd(nc, in_maps, list(range(NCORES)), **kwargs)
    if _profile is not None:
        _CACHE["last_results"] = res

    vals = np.stack([r["out_val"] for r in res.results])   # [8, B, 128, G]
    # flat partition id per (b,g): core*128 + p ; covers faces
    # core*FC + ft*128 + p for ft in 0..13
    flat = vals.transpose(1, 3, 0, 2).reshape(B, G, NCORES * 128)
    out = np.empty((B, G), np.float32)
    ftv = np.arange(FTILES)[None, None, :]
    for b in range(B):
        top = np.argpartition(flat[b], TOPM, axis=1)[:, :TOPM]   # [G, M]
        c_core = top[:, :, None] // 128
        c_part = top[:, :, None] % 128
        local = (ftv * 128 + c_part).reshape(G, TOPM * FTILES)
        cand = (c_core * FC + ftv * 128 + c_part).reshape(G, TOPM * FTILES)
        cand = np.minimum(cand, F - 1)
        dref = _d2_exact64_cand(batch_garment_verts[b], batch_body_verts[b],
                                body_faces, cand)
        dref = np.where(local >= FC, np.inf, dref)
        mn = dref.min(axis=1, keepdims=True)
        sel = np.where(dref == mn, cand, F + 1)
        tri = sel.min(axis=1)
        out[b] = _host_finish(batch_garment_verts[b], batch_body_verts[b],
                              body_faces, tri)
    return out


# revision 12
# speedup vs baseline: 2.4309x; 1.0016x over previous
"""Accurate SDF (garment-to-body signed distance) on 8 Trainium2 cores — v2.

Faces sharded 8 ways (1722/core, padded to 14*128); every core scores all
B*G garment points against its faces and returns per-PSUM-partition running
minima [B, 128, G] (no on-device argmin). Host takes the top-M partitions
per point by device score, exactly re-ranks their 14 faces each in fp64,
and finishes (region code, normals, sign) with the reference formulas.

Device math per (face f, point g), with faces on partitions and g on the
free dim (moving rows P5 = [px, py, pz, 1, |p|^2]):
  edge e (seg anchor v_e, unit dir u_e, length L_e):
    U_e = u_e.(p - v_e)                (fp32 matmul)
    T_e = clamp(U_e, 0, L_e)           (relu on Act + min on DVE/Pool)
    w_e = T_e*(2U_e - T_e)             so d2_e = |p - v_e|^2 - w_e
  A    = |p - a|^2                     (fp32 matmul, |p|^2 row)
  A_b  = A + D',  D' = -2 L_ab U_ab + L_ab^2   (Act scale/bias from U_ab)
  face: h = n^.(p - a)  (fp32 matmul), score h^2, masked by the sign of
    vb, vc, va = den - vb - vc (row-normalized fp32r matmuls) via a
    BIG*relu(-min(...)) penalty.
  sc = min(A - max(w_ab, w_ca), A_b - w_bc, h^2 + penalty)
  best[partition] = min over ft tiles  ->  DMA out per (b, gchunk).
"""

import numpy as np

B, G, V, F = 2, 1024, 6890, 13776
NCORES = 8
FC = F // NCORES            # 1722 faces per core
FTILES = 14                 # ceil(1722/128)
FPAD = FTILES * 128         # 1792
GCHUNK = 512
NMM5 = 5                    # fp32 matmuls: U_ab, U_ca, U_bc, A, h
NMM3 = 3                    # fp32r matmuls: vb, vc, va
NCST = 5                    # ptr consts: L_ab, L_ca, L_bc, -2L_ab, L_ab^2
W5COLS = B * FTILES * NMM5 * 128
W3COLS = B * FTILES * NMM3 * 128
CSTCOLS = B * FTILES * NCST
BIG = np.float32(1e6)
INF = np.float32(3e38)
TOPM = 16                   # host: partitions re-ranked exactly per point

_CACHE = {}


def _build_bass():
    import concourse.bass as bass
    import concourse.bacc as bacc
    import concourse.mybir as mybir
    from concourse.tile import TileContext

    dt = mybir.dt.float32
    dtr = mybir.dt.float32r
    Alu = mybir.AluOpType
    Act = mybir.ActivationFunctionType

    nc = bacc.Bacc()

    w5_d = nc.declare_dram_parameter("w5", [NMM5, W5COLS], dt, isOutput=False)
    w3_d = nc.declare_dram_parameter("w3r", [NMM5, W3COLS], dtr, isOutput=False)
    p5_d = nc.declare_dram_parameter("p5", [NMM5, B * G], dt, isOutput=False)
    p5r_d = nc.declare_dram_parameter("p5r", [NMM5, B * G], dtr, isOutput=False)
    cst_d = nc.declare_dram_parameter("cst", [128, CSTCOLS], dt, isOutput=False)
    oval_d = nc.declare_dram_parameter("out_val", [B, 128, G], dt, isOutput=True)

    Vv = nc.vector
    Gg = nc.gpsimd
    Ss = nc.scalar
    Tt = nc.tensor
    Sy = nc.sync

    with TileContext(nc) as tc:
        with (
            tc.tile_pool(name="cpool", bufs=1) as cpool,
            tc.tile_pool(name="work", bufs=1) as work,
            tc.tile_pool(name="acc", bufs=2) as acc,
            tc.tile_pool(name="mm", bufs=1, space="PSUM") as mm,
        ):
            w5_s = cpool.tile([NMM5, W5COLS], dt, name="w5_s")
            w3_s = cpool.tile([NMM5, W3COLS], dtr, name="w3_s")
            p5_s = cpool.tile([NMM5, B * G], dt, name="p5_s")
            p5r_s = cpool.tile([NMM5, B * G], dtr, name="p5r_s")
            cst_s = cpool.tile([128, CSTCOLS], dt, name="cst_s")
            Sy.dma_start(w5_s[:], w5_d[:])
            Sy.dma_start(w3_s[:], w3_d[:])
            Sy.dma_start(p5_s[:], p5_d[:])
            Sy.dma_start(p5r_s[:], p5r_d[:])
            Sy.dma_start(cst_s[:], cst_d[:])

            def W5(b, ft, m):
                c = ((b * FTILES + ft) * NMM5 + m) * 128
                return w5_s[:, c:c + 128]

            def W3(b, ft, m):
                c = ((b * FTILES + ft) * NMM3 + m) * 128
                return w3_s[:, c:c + 128]

            def CST(b, ft, j):
                c = (b * FTILES + ft) * NCST + j
                return cst_s[:, c:c + 1]

            sh = [128, GCHUNK]
            for b in range(B):
                for gc in range(G // GCHUNK):
                    g0 = b * G + gc * GCHUNK
                    P = p5_s[:, g0:g0 + GCHUNK]
                    Pr = p5r_s[:, g0:g0 + GCHUNK]
                    best = acc.tile(sh, dt, name="best")
                    Vv.memset(best[:], INF)
                    for ft in range(FTILES):
                        u_ab = mm.tile(sh, dt, name="u_ab")
                        u_ca = mm.tile(sh, dt, name="u_ca")
                        u_bc = mm.tile(sh, dt, name="u_bc")
                        am = mm.tile(sh, dt, name="am")
                        hm = mm.tile(sh, dt, name="hm")
                        vbm = mm.tile(sh, dt, name="vbm")
                        vcm = mm.tile(sh, dt, name="vcm")
                        vam = mm.tile(sh, dt, name="vam")
                        Tt.matmul(u_ab[:], W5(b, ft, 0), P, start=True, stop=True)
                        Tt.matmul(u_ca[:], W5(b, ft, 1), P, start=True, stop=True)
                        Tt.matmul(u_bc[:], W5(b, ft, 2), P, start=True, stop=True)
                        Tt.matmul(am[:], W5(b, ft, 3), P, start=True, stop=True)
                        Tt.matmul(hm[:], W5(b, ft, 4), P, start=True, stop=True)
                        Tt.matmul(vbm[:], W3(b, ft, 0), Pr, start=True, stop=True)
                        Tt.matmul(vcm[:], W3(b, ft, 1), Pr, start=True, stop=True)
                        Tt.matmul(vam[:], W3(b, ft, 2), Pr, start=True, stop=True)
                        # Act: drain psum fast
                        r1ab = work.tile(sh, dt, name="r1ab")
                        r1ca = work.tile(sh, dt, name="r1ca")
                        r1bc = work.tile(sh, dt, name="r1bc")
                        sf = work.tile(sh, dt, name="sf", bufs=2)
                        dp = work.tile(sh, dt, name="dp", bufs=2)
                        a_s = work.tile(sh, dt, name="a_s", bufs=2)
                        Ss.activation(r1ab[:], u_ab[:], Act.Relu)
                        Ss.activation(r1ca[:], u_ca[:], Act.Relu)
                        Ss.activation(r1bc[:], u_bc[:], Act.Relu)
                        Ss.activation(sf[:], hm[:], Act.Square)
                        Ss.activation(dp[:], u_ab[:], Act.Identity,
                                      bias=CST(b, ft, 4), scale=CST(b, ft, 3))
                        Ss.activation(a_s[:], am[:], Act.Identity)
                        # clamp T = min(relu(U), L)
                        t_ab = work.tile(sh, dt, name="t_ab")
                        t_ca = work.tile(sh, dt, name="t_ca")
                        t_bc = work.tile(sh, dt, name="t_bc")
                        Vv.tensor_scalar(t_ab[:], r1ab[:], CST(b, ft, 0), None, Alu.min)
                        Vv.tensor_scalar(t_ca[:], r1ca[:], CST(b, ft, 1), None, Alu.min)
                        Vv.tensor_scalar(t_bc[:], r1bc[:], CST(b, ft, 2), None, Alu.min)
                        # z = 2*relu(U) - T  (== 2U - T wherever T != 0)
                        z_ab = work.tile(sh, dt, name="z_ab")
                        z_ca = work.tile(sh, dt, name="z_ca")
                        z_bc = work.tile(sh, dt, name="z_bc")
                        Vv.scalar_tensor_tensor(z_ab[:], r1ab[:], 2.0, t_ab[:],
                                                Alu.mult, Alu.subtract)
                        Vv.scalar_tensor_tensor(z_ca[:], r1ca[:], 2.0, t_ca[:],
                                                Alu.mult, Alu.subtract)
                        Vv.scalar_tensor_tensor(z_bc[:], r1bc[:], 2.0, t_bc[:],
                                                Alu.mult, Alu.subtract)
                        w_ab = work.tile(sh, dt, name="w_ab", bufs=2)
                        w_ca = work.tile(sh, dt, name="w_ca", bufs=2)
                        w_bc = work.tile(sh, dt, name="w_bc", bufs=2)
                        Gg.tensor_tensor(w_ab[:], t_ab[:], z_ab[:], Alu.mult)
                        Gg.tensor_tensor(w_ca[:], t_ca[:], z_ca[:], Alu.mult)
                        Gg.tensor_tensor(w_bc[:], t_bc[:], z_bc[:], Alu.mult)
                        # face mask: penalty = BIG*relu(-min(vb,vc,va))
                        mn1 = work.tile(sh, dt, name="mn1")
                        mn2 = work.tile(sh, dt, name="mn2")
                        rneg = work.tile(sh, dt, name="rneg")
                        sfm = work.tile(sh, dt, name="sfm", bufs=2)
                        vc_s = work.tile(sh, dt, name="vc_s")
                        Ss.activation(vc_s[:], vcm[:], Act.Identity)
                        Vv.tensor_tensor(mn1[:], vbm[:], vc_s[:], Alu.min)
                        Vv.tensor_tensor(mn2[:], mn1[:], vam[:], Alu.min)
                        Ss.activation(rneg[:], mn2[:], Act.Relu, scale=-1.0)
                        Vv.scalar_tensor_tensor(sfm[:], rneg[:], float(BIG), sf[:],
                                                Alu.mult, Alu.add)
                        # combine
                        mx = work.tile(sh, dt, name="mx", bufs=2)
                        e1 = work.tile(sh, dt, name="e1", bufs=2)
                        tb2 = work.tile(sh, dt, name="tb2")
                        e2 = work.tile(sh, dt, name="e2", bufs=2)
                        em = work.tile(sh, dt, name="em", bufs=2)
                        sc = work.tile(sh, dt, name="sc")
                        Vv.tensor_tensor(mx[:], w_ab[:], w_ca[:], Alu.max)
                        Gg.tensor_tensor(e1[:], a_s[:], mx[:], Alu.subtract)
                        Gg.tensor_tensor(tb2[:], dp[:], w_bc[:], Alu.subtract)
                        Gg.tensor_tensor(e2[:], a_s[:], tb2[:], Alu.add)
                        Vv.tensor_tensor(em[:], e1[:], e2[:], Alu.min)
                        Vv.tensor_tensor(sc[:], em[:], sfm[:], Alu.min)
                        Vv.tensor_tensor(best[:], best[:], sc[:], Alu.min)
                    gg0 = gc * GCHUNK
                    Sy.dma_start(oval_d[b, :, gg0:gg0 + GCHUNK], best[:])
    nc.finalize()
    return nc


def _get_nc():
    if "nc" not in _CACHE:
        _CACHE["nc"] = _build_bass()
    return _CACHE["nc"]


def _round_fp32r(x):
    """Round fp32 -> fp32r container (11-bit mantissa, RNE)."""
    u = np.ascontiguousarray(x, np.float32).view(np.uint32)
    base = u & np.uint32(0xFFFFF000)
    low = u & np.uint32(0x00000FFF)
    half = np.uint32(0x800)
    lsb = (base >> np.uint32(12)) & np.uint32(1)
    up = (low > half) | ((low == half) & (lsb == 1))
    return np.where(up, base + np.uint32(0x1000), base).view(np.float32)


def _core_inputs(batch_garment_verts, batch_body_verts, body_faces):
    f32 = np.float32
    gv = batch_garment_verts.astype(f32)
    p5 = np.concatenate(
        [gv.transpose(0, 2, 1),                       # [B,3,G]
         np.ones((B, 1, G), f32),
         np.sum(gv * gv, -1, dtype=f32)[:, None, :]], 1
    ).reshape(B * NMM5, G)
    p5 = np.ascontiguousarray(
        p5.reshape(B, NMM5, G).transpose(1, 0, 2)).reshape(NMM5, B * G)
    p5r = _round_fp32r(p5)

    in_maps = []
    for c in range(NCORES):
        sl = slice(c * FC, (c + 1) * FC)
        w5 = np.zeros((NMM5, W5COLS), f32)
        w3 = np.zeros((NMM5, W3COLS), f32)
        cst = np.zeros((128, CSTCOLS), f32)
        for b in range(B):
            fv = batch_body_verts[b].astype(f32)[body_faces[sl]]  # [FC,3,3]
            a, bb, cc = fv[:, 0], fv[:, 1], fv[:, 2]
            n = np.cross((bb - a).astype(np.float64),
                         (cc - a).astype(np.float64))
            nn = np.linalg.norm(n, axis=1)
            ab, ac = (bb - a).astype(f32), (cc - a).astype(f32)
            naa = np.sum(ab * ab, -1, dtype=f32)
            nab = np.sum(ab * ac, -1, dtype=f32)
            ncc = np.sum(ac * ac, -1, dtype=f32)
            den = (naa * ncc - nab * nab).astype(f32)
            degen = (den < f32(1e-4)) | (nn < 1e-10)
            nh = np.where(degen[:, None], 0,
                          n / np.maximum(nn, 1e-30)[:, None]).astype(f32)

            def pads(rows):  # [FC,5] -> [5, FPAD] padded
                out = np.zeros((NMM5, FPAD), f32)
                out[:, :FC] = rows.T
                return out

            Ls = {}
            Ws = {}
            for nm, (ea, eb) in (("ab", (a, bb)), ("ca", (a, cc)),
                                 ("bc", (bb, cc))):
                ed = (eb - ea).astype(f32)
                L = np.linalg.norm(ed.astype(np.float64), axis=1).astype(f32)
                u = np.where(L[:, None] > 0,
                             ed / np.maximum(L, f32(1e-30))[:, None], 0)
                Ws[nm] = pads(np.concatenate(
                    [u, (-np.sum(u * ea, -1, dtype=f32))[:, None],
                     np.zeros((FC, 1), f32)], 1))
                Lp = np.zeros(FPAD, f32)
                Lp[:FC] = L
                Ls[nm] = Lp
            wa_rows = np.concatenate(
                [-2 * a, np.sum(a * a, -1, dtype=f32)[:, None],
                 np.ones((FC, 1), f32)], 1)
            WA = pads(wa_rows)
            WA[3, FC:] = f32(1e30)                       # pad faces: huge A
            Wh = pads(np.concatenate(
                [nh, (-np.sum(nh * a, -1, dtype=f32))[:, None],
                 np.zeros((FC, 1), f32)], 1))
            wvb = (ncc[:, None] * ab - nab[:, None] * ac).astype(f32)
            wvc = (naa[:, None] * ac - nab[:, None] * ab).astype(f32)
            cvb = (-np.sum(wvb * a, -1, dtype=f32))
            cvc = (-np.sum(wvc * a, -1, dtype=f32))
            wva = (-(wvb + wvc)).astype(f32)
            cva = (den - cvb - cvc).astype(f32)
            W3L = []
            for w_, c_ in ((wvb, cvb), (wvc, cvc), (wva, cva)):
                s = np.maximum(np.linalg.norm(w_, axis=1), f32(1e-30)).astype(f32)
                W3L.append(pads(np.concatenate(
                    [w_ / s[:, None], (c_ / s)[:, None],
                     np.zeros((FC, 1), f32)], 1)))
            Wvb, Wvc, Wva = W3L
            Wva[:, :FC][:, degen] = 0.0
            Wva[3, :FC][degen] = -1.0
            Wva[:, FC:] = 0.0
            Wva[3, FC:] = -1.0                           # pad faces: outside

            mm5 = [Ws["ab"], Ws["ca"], Ws["bc"], WA, Wh]
            mm3 = [Wvb, Wvc, Wva]
            for ft in range(FTILES):
                fsl = slice(ft * 128, (ft + 1) * 128)
                for m in range(NMM5):
                    c0 = ((b * FTILES + ft) * NMM5 + m) * 128
                    w5[:, c0:c0 + 128] = mm5[m][:, fsl]
                for m in range(NMM3):
                    c0 = ((b * FTILES + ft) * NMM3 + m) * 128
                    w3[:, c0:c0 + 128] = mm3[m][:, fsl]
                c0 = (b * FTILES + ft) * NCST
                cst[:, c0 + 0] = Ls["ab"][fsl]
                cst[:, c0 + 1] = Ls["ca"][fsl]
                cst[:, c0 + 2] = Ls["bc"][fsl]
                cst[:, c0 + 3] = -2 * Ls["ab"][fsl]
                cst[:, c0 + 4] = Ls["ab"][fsl] ** 2
        in_maps.append({"w5": w5, "w3r": _round_fp32r(w3),
                        "p5": p5, "p5r": p5r, "cst": cst})
    return in_maps


def _d2_exact64_cand(p, bverts, faces, cand):
    """Exact fp64 point-triangle dist^2 for candidate faces. cand [G,C]."""
    fv = bverts[faces[cand]].astype(np.float64)      # [G,C,3,3]
    a, b, c = fv[:, :, 0], fv[:, :, 1], fv[:, :, 2]
    q = p.astype(np.float64)[:, None, :]
    best = np.full(cand.shape, np.inf)
    for ea, eb in ((a, b), (b, c), (c, a)):
        ed = eb - ea
        L2 = np.sum(ed * ed, -1)
        pe = q - ea
        t = np.clip(np.sum(pe * ed, -1) / np.maximum(L2, 1e-300), 0, 1)
        d = pe - t[..., None] * ed
        best = np.minimum(best, np.sum(d * d, -1))
    ab, ac = b - a, c - a
    n = np.cross(ab, ac)
    naa = np.sum(ab * ab, -1); nab = np.sum(ab * ac, -1)
    ncc = np.sum(ac * ac, -1)
    den = naa * ncc - nab * nab
    pa = q - a
    d1 = np.sum(pa * ab, -1); d2_ = np.sum(pa * ac, -1)
    vb = ncc * d1 - nab * d2_; vc = naa * d2_ - nab * d1
    va = den - vb - vc
    inside = (vb >= 0) & (vc >= 0) & (va >= 0) & (den > 1e-300)
    hn = np.sum(pa * n, -1)
    h2 = hn * hn / np.maximum(den, 1e-300)
    return np.where(inside, np.minimum(best, h2), best)


def _host_finish(g_verts, b_verts, faces, tri):
    """Exact reference finish for the winning face of each garment point."""
    f32 = np.float32
    EPS = f32(1e-10)

    def safe(x):
        return np.where(np.abs(x) < 1e-12, f32(1e-12), x).astype(f32)

    fverts = b_verts[faces]
    a_, b_, c_ = fverts[:, 0], fverts[:, 1], fverts[:, 2]
    fn_raw = np.cross(b_ - a_, c_ - a_).astype(f32)
    vn = np.zeros_like(b_verts)
    for k in range(3):
        np.add.at(vn, faces[:, k], fn_raw)
    vn = vn / (np.linalg.norm(vn, axis=-1, keepdims=True).astype(f32) + EPS)
    fn = fn_raw / (np.linalg.norm(fn_raw, axis=-1, keepdims=True).astype(f32) + EPS)

    a = a_[tri]; bb = b_[tri]; cc = c_[tri]
    q = g_verts
    ab = bb - a; ac = cc - a
    ap = q - a
    d1 = np.sum(ab * ap, -1); d2 = np.sum(ac * ap, -1)
    bp = q - bb
    d3 = np.sum(ab * bp, -1); d4 = np.sum(ac * bp, -1)
    cp = q - cc
    d5 = np.sum(ab * cp, -1); d6 = np.sum(ac * cp, -1)
    vc = d1 * d4 - d3 * d2
    vb = d5 * d2 - d1 * d6
    va = d3 * d6 - d5 * d4
    denom = safe(va + vb + vc)
    v, w = (vb / denom).astype(f32), (vc / denom).astype(f32)
    part = np.zeros(v.shape, np.int32)
    t_bc = ((d4 - d3) / safe((d4 - d3) + (d5 - d6))).astype(f32)
    m = (va <= 0) & (d4 - d3 >= 0) & (d5 - d6 >= 0)
    v = np.where(m, 1.0 - t_bc, v).astype(f32)
    w = np.where(m, t_bc, w).astype(f32)
    part = np.where(m, 2, part)
    t_ac = (d2 / safe(d2 - d6)).astype(f32)
    m = (vb <= 0) & (d2 >= 0) & (d6 <= 0)
    v = np.where(m, 0.0, v).astype(f32)
    w = np.where(m, t_ac, w).astype(f32)
    part = np.where(m, 3, part)
    m = (d6 >= 0) & (d5 <= d6)
    v = np.where(m, 0.0, v).astype(f32)
    w = np.where(m, 1.0, w).astype(f32)
    part = np.where(m, 6, part)
    t_ab = (d1 / safe(d1 - d3)).astype(f32)
    m = (vc <= 0) & (d1 >= 0) & (d3 <= 0)
    v = np.where(m, t_ab, v).astype(f32)
    w = np.where(m, 0.0, w).astype(f32)
    part = np.where(m, 1, part)
    m = (d3 >= 0) & (d4 <= d3)
    v = np.where(m, 1.0, v).astype(f32)
    w = np.where(m, 0.0, w).astype(f32)
    part = np.where(m, 5, part)
    m = (d1 <= 0) & (d2 <= 0)
    v = np.where(m, 0.0, v).astype(f32)
    w = np.where(m, 0.0, w).astype(f32)
    part = np.where(m, 4, part)
    npt = a + v[:, None] * ab + w[:, None] * ac

    fidx = faces[tri]
    gar = np.arange(len(tri))
    take = lambda col: vn[fidx[gar, col]]
    n_face = fn[tri]
    n_vert = take(np.clip(part - 4, 0, 2))
    n_edge = take(np.clip(part - 1, 0, 2)) + take(np.mod(part, 3))
    n = np.where((part == 0)[:, None], n_face,
                 np.where((part > 3)[:, None], n_vert, n_edge)).astype(f32)
    n = n / (np.linalg.norm(n, axis=-1, keepdims=True).astype(f32) + EPS)
    return np.sum((g_verts - npt) * n, axis=1).astype(f32)


def kernel(batch_garment_verts, batch_body_verts, body_faces, _profile=None):
    from concourse.bass_utils import run_bass_kernel_spmd

    batch_garment_verts = np.asarray(batch_garment_verts, dtype=np.float32)
    batch_body_verts = np.asarray(batch_body_verts, dtype=np.float32)
    body_faces = np.asarray(body_faces)

    nc = _get_nc()
    in_maps = _core_inputs(batch_garment_verts, batch_body_verts, body_faces)
    kwargs = dict(_profile) if _profile else {}
    res = run_bass_kernel_spmd(nc, in_maps, list(range(NCORES)), **kwargs)
    if _profile is not None:
        _CACHE["last_results"] = res

    vals = np.stack([r["out_val"] for r in res.results])   # [8, B, 128, G]
    # flat partition id per (b,g): core*128 + p ; covers faces
    # core*FC + ft*128 + p for ft in 0..13
    flat = vals.transpose(1, 3, 0, 2).reshape(B, G, NCORES * 128)
    out = np.empty((B, G), np.float32)
    ftv = np.arange(FTILES)[None, None, :]
    for b in range(B):
        top = np.argpartition(flat[b], TOPM, axis=1)[:, :TOPM]   # [G, M]
        c_core = top[:, :, None] // 128
        c_part = top[:, :, None] % 128
        local = (ftv * 128 + c_part).reshape(G, TOPM * FTILES)
        cand = (c_core * FC + ftv * 128 + c_part).reshape(G, TOPM * FTILES)
        cand = np.minimum(cand, F - 1)
        dref = _d2_exact64_cand(batch_garment_verts[b], batch_body_verts[b],
                                body_faces, cand)
        dref = np.where(local >= FC, np.inf, dref)
        mn = dref.min(axis=1, keepdims=True)
        sel = np.where(dref == mn, cand, F + 1)
        tri = sel.min(axis=1)
        out[b] = _host_finish(batch_garment_verts[b], batch_body_verts[b],
                              body_faces, tri)
    return out


# revision 14
# speedup vs baseline: 2.4781x; 1.0194x over previous
"""Accurate SDF (garment-to-body signed distance) on 8 Trainium2 cores — v2.

Faces sharded 8 ways (1722/core, padded to 14*128); every core scores all
B*G garment points against its faces and returns per-PSUM-partition running
minima [B, 128, G] (no on-device argmin). Host takes the top-M partitions
per point by device score, exactly re-ranks their 14 faces each in fp64,
and finishes (region code, normals, sign) with the reference formulas.

Device math per (face f, point g), with faces on partitions and g on the
free dim (moving rows P5 = [px, py, pz, 1, |p|^2]):
  edge e (seg anchor v_e, unit dir u_e, length L_e):
    U_e = u_e.(p - v_e)                (fp32 matmul)
    T_e = clamp(U_e, 0, L_e)           (relu on Act + min on DVE/Pool)
    w_e = T_e*(2U_e - T_e)             so d2_e = |p - v_e|^2 - w_e
  A    = |p - a|^2                     (fp32 matmul, |p|^2 row)
  A_b  = A + D',  D' = -2 L_ab U_ab + L_ab^2   (Act scale/bias from U_ab)
  face: h = n^.(p - a)  (fp32 matmul), score h^2, masked by the sign of
    vb, vc, va = den - vb - vc (row-normalized fp32r matmuls) via a
    BIG*relu(-min(...)) penalty.
  sc = min(A - max(w_ab, w_ca), A_b - w_bc, h^2 + penalty)
  best[partition] = min over ft tiles  ->  DMA out per (b, gchunk).
"""

import numpy as np

B, G, V, F = 2, 1024, 6890, 13776
NCORES = 8
FC = F // NCORES            # 1722 faces per core
FTILES = 14                 # ceil(1722/128)
FPAD = FTILES * 128         # 1792
GCHUNK = 512
NMM5 = 5                    # fp32 matmuls: U_ab, U_ca, U_bc, A, h
NMM3 = 3                    # fp32r matmuls: vb, vc, va
NCST = 5                    # ptr consts: L_ab, L_ca, L_bc, -2L_ab, L_ab^2
W5COLS = B * FTILES * NMM5 * 128
W3COLS = B * FTILES * NMM3 * 128
CSTCOLS = B * FTILES * NCST
BIG = np.float32(1e6)
INF = np.float32(3e38)
TOPM = 16                   # host: partitions re-ranked exactly per point

_CACHE = {}


def _build_bass():
    import concourse.bass as bass
    import concourse.bacc as bacc
    import concourse.mybir as mybir
    from concourse.tile import TileContext

    dt = mybir.dt.float32
    dtr = mybir.dt.float32r
    Alu = mybir.AluOpType
    Act = mybir.ActivationFunctionType

    nc = bacc.Bacc()

    w5_d = nc.declare_dram_parameter("w5", [NMM5, W5COLS], dt, isOutput=False)
    w3_d = nc.declare_dram_parameter("w3r", [NMM5, W3COLS], dtr, isOutput=False)
    p5_d = nc.declare_dram_parameter("p5", [NMM5, B * G], dt, isOutput=False)
    p5r_d = nc.declare_dram_parameter("p5r", [NMM5, B * G], dtr, isOutput=False)
    cst_d = nc.declare_dram_parameter("cst", [128, CSTCOLS], dt, isOutput=False)
    oval_d = nc.declare_dram_parameter("out_val", [B, 128, G], dt, isOutput=True)

    Vv = nc.vector
    Gg = nc.gpsimd
    Ss = nc.scalar
    Tt = nc.tensor
    Sy = nc.sync

    with TileContext(nc) as tc:
        with (
            tc.tile_pool(name="cpool", bufs=1) as cpool,
            tc.tile_pool(name="work", bufs=1) as work,
            tc.tile_pool(name="acc", bufs=2) as acc,
            tc.tile_pool(name="mm", bufs=1, space="PSUM") as mm,
        ):
            w5_s = cpool.tile([NMM5, W5COLS], dt, name="w5_s")
            w3_s = cpool.tile([NMM5, W3COLS], dtr, name="w3_s")
            p5_s = cpool.tile([NMM5, B * G], dt, name="p5_s")
            p5r_s = cpool.tile([NMM5, B * G], dtr, name="p5r_s")
            cst_s = cpool.tile([128, CSTCOLS], dt, name="cst_s")
            Sy.dma_start(w5_s[:], w5_d[:])
            Sy.dma_start(w3_s[:], w3_d[:])
            Sy.dma_start(p5_s[:], p5_d[:])
            Sy.dma_start(p5r_s[:], p5r_d[:])
            Sy.dma_start(cst_s[:], cst_d[:])

            def W5(b, ft, m):
                c = ((b * FTILES + ft) * NMM5 + m) * 128
                return w5_s[:, c:c + 128]

            def W3(b, ft, m):
                c = ((b * FTILES + ft) * NMM3 + m) * 128
                return w3_s[:, c:c + 128]

            def CST(b, ft, j):
                c = (b * FTILES + ft) * NCST + j
                return cst_s[:, c:c + 1]

            sh = [128, GCHUNK]
            for b in range(B):
                for gc in range(G // GCHUNK):
                    g0 = b * G + gc * GCHUNK
                    P = p5_s[:, g0:g0 + GCHUNK]
                    Pr = p5r_s[:, g0:g0 + GCHUNK]
                    best = acc.tile(sh, dt, name="best")
                    Vv.memset(best[:], INF)
                    for ft in range(FTILES):
                        u_ab = mm.tile(sh, dt, name="u_ab")
                        u_ca = mm.tile(sh, dt, name="u_ca")
                        u_bc = mm.tile(sh, dt, name="u_bc")
                        am = mm.tile(sh, dt, name="am")
                        hm = mm.tile(sh, dt, name="hm")
                        vbm = mm.tile(sh, dt, name="vbm")
                        vcm = mm.tile(sh, dt, name="vcm")
                        vam = mm.tile(sh, dt, name="vam")
                        Tt.matmul(u_ab[:], W5(b, ft, 0), P, start=True, stop=True)
                        Tt.matmul(u_ca[:], W5(b, ft, 1), P, start=True, stop=True)
                        Tt.matmul(u_bc[:], W5(b, ft, 2), P, start=True, stop=True)
                        Tt.matmul(am[:], W5(b, ft, 3), P, start=True, stop=True)
                        Tt.matmul(hm[:], W5(b, ft, 4), P, start=True, stop=True)
                        Tt.matmul(vbm[:], W3(b, ft, 0), Pr, start=True, stop=True)
                        Tt.matmul(vcm[:], W3(b, ft, 1), Pr, start=True, stop=True)
                        Tt.matmul(vam[:], W3(b, ft, 2), Pr, start=True, stop=True)
                        # Act: drain psum fast
                        r1ab = work.tile(sh, dt, name="r1ab")
                        r1ca = work.tile(sh, dt, name="r1ca")
                        r1bc = work.tile(sh, dt, name="r1bc")
                        sf = work.tile(sh, dt, name="sf", bufs=2)
                        dp = work.tile(sh, dt, name="dp", bufs=2)
                        a_s = work.tile(sh, dt, name="a_s", bufs=2)
                        Ss.activation(r1ab[:], u_ab[:], Act.Relu)
                        Ss.activation(r1ca[:], u_ca[:], Act.Relu)
                        Ss.activation(r1bc[:], u_bc[:], Act.Relu)
                        Ss.activation(sf[:], hm[:], Act.Square)
                        Ss.activation(dp[:], u_ab[:], Act.Identity,
                                      bias=CST(b, ft, 4), scale=CST(b, ft, 3))
                        Ss.activation(a_s[:], am[:], Act.Identity)
                        # clamp T = min(relu(U), L)
                        t_ab = work.tile(sh, dt, name="t_ab")
                        t_ca = work.tile(sh, dt, name="t_ca")
                        t_bc = work.tile(sh, dt, name="t_bc")
                        Vv.tensor_scalar(t_ab[:], r1ab[:], CST(b, ft, 0), None, Alu.min)
                        Vv.tensor_scalar(t_ca[:], r1ca[:], CST(b, ft, 1), None, Alu.min)
                        Vv.tensor_scalar(t_bc[:], r1bc[:], CST(b, ft, 2), None, Alu.min)
                        # z = 2*relu(U) - T  (== 2U - T wherever T != 0)
                        z_ab = work.tile(sh, dt, name="z_ab")
                        z_ca = work.tile(sh, dt, name="z_ca")
                        z_bc = work.tile(sh, dt, name="z_bc")
                        Vv.scalar_tensor_tensor(z_ab[:], r1ab[:], 2.0, t_ab[:],
                                                Alu.mult, Alu.subtract)
                        Vv.scalar_tensor_tensor(z_ca[:], r1ca[:], 2.0, t_ca[:],
                                                Alu.mult, Alu.subtract)
                        Vv.scalar_tensor_tensor(z_bc[:], r1bc[:], 2.0, t_bc[:],
                                                Alu.mult, Alu.subtract)
                        w_ab = work.tile(sh, dt, name="w_ab", bufs=2)
                        w_ca = work.tile(sh, dt, name="w_ca")
                        w_bc = work.tile(sh, dt, name="w_bc")
                        Gg.tensor_tensor(w_ab[:], t_ab[:], z_ab[:], Alu.mult)
                        Gg.tensor_tensor(w_ca[:], t_ca[:], z_ca[:], Alu.mult)
                        Gg.tensor_tensor(w_bc[:], t_bc[:], z_bc[:], Alu.mult)
                        # face mask: penalty = BIG*relu(-min(vb,vc,va))
                        mn1 = work.tile(sh, dt, name="mn1")
                        mn2 = work.tile(sh, dt, name="mn2")
                        rneg = work.tile(sh, dt, name="rneg")
                        sfm = work.tile(sh, dt, name="sfm", bufs=2)
                        vb_s = work.tile(sh, dt, name="vb_s")
                        vc_s = work.tile(sh, dt, name="vc_s")
                        va_s = work.tile(sh, dt, name="va_s")
                        Ss.activation(vb_s[:], vbm[:], Act.Identity)
                        Ss.activation(vc_s[:], vcm[:], Act.Identity)
                        Ss.activation(va_s[:], vam[:], Act.Identity)
                        Vv.tensor_tensor(mn1[:], vb_s[:], vc_s[:], Alu.min)
                        Vv.tensor_tensor(mn2[:], mn1[:], va_s[:], Alu.min)
                        Ss.activation(rneg[:], mn2[:], Act.Relu, scale=-1.0)
                        Vv.scalar_tensor_tensor(sfm[:], rneg[:], float(BIG), sf[:],
                                                Alu.mult, Alu.add)
                        # combine
                        mx = work.tile(sh, dt, name="mx", bufs=2)
                        e1 = work.tile(sh, dt, name="e1", bufs=2)
                        tb2 = work.tile(sh, dt, name="tb2")
                        e2 = work.tile(sh, dt, name="e2", bufs=2)
                        em = work.tile(sh, dt, name="em", bufs=2)
                        sc = work.tile(sh, dt, name="sc")
                        Vv.tensor_tensor(mx[:], w_ab[:], w_ca[:], Alu.max)
                        Gg.tensor_tensor(e1[:], a_s[:], mx[:], Alu.subtract)
                        Gg.tensor_tensor(tb2[:], dp[:], w_bc[:], Alu.subtract)
                        Gg.tensor_tensor(e2[:], a_s[:], tb2[:], Alu.add)
                        Vv.tensor_tensor(em[:], e1[:], e2[:], Alu.min)
                        Vv.tensor_tensor(sc[:], em[:], sfm[:], Alu.min)
                        Vv.tensor_tensor(best[:], best[:], sc[:], Alu.min)
                    gg0 = gc * GCHUNK
                    Sy.dma_start(oval_d[b, :, gg0:gg0 + GCHUNK], best[:])
    nc.finalize()
    return nc


def _get_nc():
    if "nc" not in _CACHE:
        _CACHE["nc"] = _build_bass()
    return _CACHE["nc"]


def _round_fp32r(x):
    """Round fp32 -> fp32r container (11-bit mantissa, RNE)."""
    u = np.ascontiguousarray(x, np.float32).view(np.uint32)
    base = u & np.uint32(0xFFFFF000)
    low = u & np.uint32(0x00000FFF)
    half = np.uint32(0x800)
    lsb = (base >> np.uint32(12)) & np.uint32(1)
    up = (low > half) | ((low == half) & (lsb == 1))
    return np.where(up, base + np.uint32(0x1000), base).view(np.float32)


def _core_inputs(batch_garment_verts, batch_body_verts, body_faces):
    f32 = np.float32
    gv = batch_garment_verts.astype(f32)
    p5 = np.concatenate(
        [gv.transpose(0, 2, 1),                       # [B,3,G]
         np.ones((B, 1, G), f32),
         np.sum(gv * gv, -1, dtype=f32)[:, None, :]], 1
    ).reshape(B * NMM5, G)
    p5 = np.ascontiguousarray(
        p5.reshape(B, NMM5, G).transpose(1, 0, 2)).reshape(NMM5, B * G)
    p5r = _round_fp32r(p5)

    in_maps = []
    for c in range(NCORES):
        sl = slice(c * FC, (c + 1) * FC)
        w5 = np.zeros((NMM5, W5COLS), f32)
        w3 = np.zeros((NMM5, W3COLS), f32)
        cst = np.zeros((128, CSTCOLS), f32)
        for b in range(B):
            fv = batch_body_verts[b].astype(f32)[body_faces[sl]]  # [FC,3,3]
            a, bb, cc = fv[:, 0], fv[:, 1], fv[:, 2]
            n = np.cross((bb - a).astype(np.float64),
                         (cc - a).astype(np.float64))
            nn = np.linalg.norm(n, axis=1)
            ab, ac = (bb - a).astype(f32), (cc - a).astype(f32)
            naa = np.sum(ab * ab, -1, dtype=f32)
            nab = np.sum(ab * ac, -1, dtype=f32)
            ncc = np.sum(ac * ac, -1, dtype=f32)
            den = (naa * ncc - nab * nab).astype(f32)
            degen = (den < f32(1e-4)) | (nn < 1e-10)
            nh = np.where(degen[:, None], 0,
                          n / np.maximum(nn, 1e-30)[:, None]).astype(f32)

            def pads(rows):  # [FC,5] -> [5, FPAD] padded
                out = np.zeros((NMM5, FPAD), f32)
                out[:, :FC] = rows.T
                return out

            Ls = {}
            Ws = {}
            for nm, (ea, eb) in (("ab", (a, bb)), ("ca", (a, cc)),
                                 ("bc", (bb, cc))):
                ed = (eb - ea).astype(f32)
                L = np.linalg.norm(ed.astype(np.float64), axis=1).astype(f32)
                u = np.where(L[:, None] > 0,
                             ed / np.maximum(L, f32(1e-30))[:, None], 0)
                Ws[nm] = pads(np.concatenate(
                    [u, (-np.sum(u * ea, -1, dtype=f32))[:, None],
                     np.zeros((FC, 1), f32)], 1))
                Lp = np.zeros(FPAD, f32)
                Lp[:FC] = L
                Ls[nm] = Lp
            wa_rows = np.concatenate(
                [-2 * a, np.sum(a * a, -1, dtype=f32)[:, None],
                 np.ones((FC, 1), f32)], 1)
            WA = pads(wa_rows)
            WA[3, FC:] = f32(1e30)                       # pad faces: huge A
            Wh = pads(np.concatenate(
                [nh, (-np.sum(nh * a, -1, dtype=f32))[:, None],
                 np.zeros((FC, 1), f32)], 1))
            wvb = (ncc[:, None] * ab - nab[:, None] * ac).astype(f32)
            wvc = (naa[:, None] * ac - nab[:, None] * ab).astype(f32)
            cvb = (-np.sum(wvb * a, -1, dtype=f32))
            cvc = (-np.sum(wvc * a, -1, dtype=f32))
            wva = (-(wvb + wvc)).astype(f32)
            cva = (den - cvb - cvc).astype(f32)
            W3L = []
            for w_, c_ in ((wvb, cvb), (wvc, cvc), (wva, cva)):
                s = np.maximum(np.linalg.norm(w_, axis=1), f32(1e-30)).astype(f32)
                W3L.append(pads(np.concatenate(
                    [w_ / s[:, None], (c_ / s)[:, None],
                     np.zeros((FC, 1), f32)], 1)))
            Wvb, Wvc, Wva = W3L
            Wva[:, :FC][:, degen] = 0.0
            Wva[3, :FC][degen] = -1.0
            Wva[:, FC:] = 0.0
            Wva[3, FC:] = -1.0                           # pad faces: outside

            mm5 = [Ws["ab"], Ws["ca"], Ws["bc"], WA, Wh]
            mm3 = [Wvb, Wvc, Wva]
            for ft in range(FTILES):
                fsl = slice(ft * 128, (ft + 1) * 128)
                for m in range(NMM5):
                    c0 = ((b * FTILES + ft) * NMM5 + m) * 128
                    w5[:, c0:c0 + 128] = mm5[m][:, fsl]
                for m in range(NMM3):
                    c0 = ((b * FTILES + ft) * NMM3 + m) * 128
                    w3[:, c0:c0 + 128] = mm3[m][:, fsl]
                c0 = (b * FTILES + ft) * NCST
                cst[:, c0 + 0] = Ls["ab"][fsl]
                cst[:, c0 + 1] = Ls["ca"][fsl]
                cst[:, c0 + 2] = Ls["bc"][fsl]
                cst[:, c0 + 3] = -2 * Ls["ab"][fsl]
                cst[:, c0 + 4] = Ls["ab"][fsl] ** 2
        in_maps.append({"w5": w5, "w3r": _round_fp32r(w3),
                        "p5": p5, "p5r": p5r, "cst": cst})
    return in_maps


def _d2_exact64_cand(p, bverts, faces, cand):
    """Exact fp64 point-triangle dist^2 for candidate faces. cand [G,C]."""
    fv = bverts[faces[cand]].astype(np.float64)      # [G,C,3,3]
    a, b, c = fv[:, :, 0], fv[:, :, 1], fv[:, :, 2]
    q = p.astype(np.float64)[:, None, :]
    best = np.full(cand.shape, np.inf)
    for ea, eb in ((a, b), (b, c), (c, a)):
        ed = eb - ea
        L2 = np.sum(ed * ed, -1)
        pe = q - ea
        t = np.clip(np.sum(pe * ed, -1) / np.maximum(L2, 1e-300), 0, 1)
        d = pe - t[..., None] * ed
        best = np.minimum(best, np.sum(d * d, -1))
    ab, ac = b - a, c - a
    n = np.cross(ab, ac)
    naa = np.sum(ab * ab, -1); nab = np.sum(ab * ac, -1)
    ncc = np.sum(ac * ac, -1)
    den = naa * ncc - nab * nab
    pa = q - a
    d1 = np.sum(pa * ab, -1); d2_ = np.sum(pa * ac, -1)
    vb = ncc * d1 - nab * d2_; vc = naa * d2_ - nab * d1
    va = den - vb - vc
    inside = (vb >= 0) & (vc >= 0) & (va >= 0) & (den > 1e-300)
    hn = np.sum(pa * n, -1)
    h2 = hn * hn / np.maximum(den, 1e-300)
    return np.where(inside, np.minimum(best, h2), best)


def _host_finish(g_verts, b_verts, faces, tri):
    """Exact reference finish for the winning face of each garment point."""
    f32 = np.float32
    EPS = f32(1e-10)

    def safe(x):
        return np.where(np.abs(x) < 1e-12, f32(1e-12), x).astype(f32)

    fverts = b_verts[faces]
    a_, b_, c_ = fverts[:, 0], fverts[:, 1], fverts[:, 2]
    fn_raw = np.cross(b_ - a_, c_ - a_).astype(f32)
    vn = np.zeros_like(b_verts)
    for k in range(3):
        np.add.at(vn, faces[:, k], fn_raw)
    vn = vn / (np.linalg.norm(vn, axis=-1, keepdims=True).astype(f32) + EPS)
    fn = fn_raw / (np.linalg.norm(fn_raw, axis=-1, keepdims=True).astype(f32) + EPS)

    a = a_[tri]; bb = b_[tri]; cc = c_[tri]
    q = g_verts
    ab = bb - a; ac = cc - a
    ap = q - a
    d1 = np.sum(ab * ap, -1); d2 = np.sum(ac * ap, -1)
    bp = q - bb
    d3 = np.sum(ab * bp, -1); d4 = np.sum(ac * bp, -1)
    cp = q - cc
    d5 = np.sum(ab * cp, -1); d6 = np.sum(ac * cp, -1)
    vc = d1 * d4 - d3 * d2
    vb = d5 * d2 - d1 * d6
    va = d3 * d6 - d5 * d4
    denom = safe(va + vb + vc)
    v, w = (vb / denom).astype(f32), (vc / denom).astype(f32)
    part = np.zeros(v.shape, np.int32)
    t_bc = ((d4 - d3) / safe((d4 - d3) + (d5 - d6))).astype(f32)
    m = (va <= 0) & (d4 - d3 >= 0) & (d5 - d6 >= 0)
    v = np.where(m, 1.0 - t_bc, v).astype(f32)
    w = np.where(m, t_bc, w).astype(f32)
    part = np.where(m, 2, part)
    t_ac = (d2 / safe(d2 - d6)).astype(f32)
    m = (vb <= 0) & (d2 >= 0) & (d6 <= 0)
    v = np.where(m, 0.0, v).astype(f32)
    w = np.where(m, t_ac, w).astype(f32)
    part = np.where(m, 3, part)
    m = (d6 >= 0) & (d5 <= d6)
    v = np.where(m, 0.0, v).astype(f32)
    w = np.where(m, 1.0, w).astype(f32)
    part = np.where(m, 6, part)
    t_ab = (d1 / safe(d1 - d3)).astype(f32)
    m = (vc <= 0) & (d1 >= 0) & (d3 <= 0)
    v = np.where(m, t_ab, v).astype(f32)
    w = np.where(m, 0.0, w).astype(f32)
    part = np.where(m, 1, part)
    m = (d3 >= 0) & (d4 <= d3)
    v = np.where(m, 1.0, v).astype(f32)
    w = np.where(m, 0.0, w).astype(f32)
    part = np.where(m, 5, part)
    m = (d1 <= 0) & (d2 <= 0)
    v = np.where(m, 0.0, v).astype(f32)
    w = np.where(m, 0.0, w).astype(f32)
    part = np.where(m, 4, part)
    npt = a + v[:, None] * ab + w[:, None] * ac

    fidx = faces[tri]
    gar = np.arange(len(tri))
    take = lambda col: vn[fidx[gar, col]]
    n_face = fn[tri]
    n_vert = take(np.clip(part - 4, 0, 2))
    n_edge = take(np.clip(part - 1, 0, 2)) + take(np.mod(part, 3))
    n = np.where((part == 0)[:, None], n_face,
                 np.where((part > 3)[:, None], n_vert, n_edge)).astype(f32)
    n = n / (np.linalg.norm(n, axis=-1, keepdims=True).astype(f32) + EPS)
    return np.sum((g_verts - npt) * n, axis=1).astype(f32)


def kernel(batch_garment_verts, batch_body_verts, body_faces, _profile=None):
    from concourse.bass_utils import run_bass_kernel_spmd

    batch_garment_verts = np.asarray(batch_garment_verts, dtype=np.float32)
    batch_body_verts = np.asarray(batch_body_verts, dtype=np.float32)
    body_faces = np.asarray(body_faces)

    nc = _get_nc()
    in_maps = _core_inputs(batch_garment_verts, batch_body_verts, body_faces)
    kwargs = dict(_profile) if _profile else {}
    res = run_bass_kernel_spmd(nc, in_maps, list(range(NCORES)), **kwargs)
    if _profile is not None:
        _CACHE["last_results"] = res

    vals = np.stack([r["out_val"] for r in res.results])   # [8, B, 128, G]
    # flat partition id per (b,g): core*128 + p ; covers faces
    # core*FC + ft*128 + p for ft in 0..13
    flat = vals.transpose(1, 3, 0, 2).reshape(B, G, NCORES * 128)
    out = np.empty((B, G), np.float32)
    ftv = np.arange(FTILES)[None, None, :]
    for b in range(B):
        top = np.argpartition(flat[b], TOPM, axis=1)[:, :TOPM]   # [G, M]
        c_core = top[:, :, None] // 128
        c_part = top[:, :, None] % 128
        local = (ftv * 128 + c_part).reshape(G, TOPM * FTILES)
        cand = (c_core * FC + ftv * 128 + c_part).reshape(G, TOPM * FTILES)
        cand = np.minimum(cand, F - 1)
        dref = _d2_exact64_cand(batch_garment_verts[b], batch_body_verts[b],
                                body_faces, cand)
        dref = np.where(local >= FC, np.inf, dref)
        mn = dref.min(axis=1, keepdims=True)
        sel = np.where(dref == mn, cand, F + 1)
        tri = sel.min(axis=1)
        out[b] = _host_finish(batch_garment_verts[b], batch_body_verts[b],
                              body_faces, tri)
    return out
